# revision 19
# baseline (speedup 1.0000x reference)
"""Trainium2 Bass kernel for nn_KernelMachine (random-feature kernel machine).

Computes out = (sqrt(2/N) * cos(x @ Wf^T + bf)) @ Wp on 8 NeuronCores,
data-parallel over the batch dimension (1024 rows/core), no collectives.

Per-core pipeline, per 128-wide tile of the N=4096 feature dim, with the
elementwise work load-balanced across DVE and the Activation engine:

  MM1 (TensorE, f32r): t = [x | 1] @ [Wf/2pi | bf']  -> PSUM (in turns)
      (bias and the cos->sin quarter-turn fold into the ones-row weights)
  then one of two per-tile paths to g ~ sin(2pi t):
   - path 'd' (DVE): one fused custom op on [128,1024]: r = t - rne(t) via
     the 1.5*2^23 magic constant, then the factored quintic r(A-r^2)(B-r^2)
     which is sin(2pi r)/K for K=53.52...  (K folds into this tile's Wp)
   - path 'h' (Act+Pool+PE, two [128,512] halves): Act u0 = t + 1.5*2^23
     (= magic + rne(t), SBUF), Pool u = u0 - magic (bf16, small ints),
     PE accumulates t - u into the same PSUM bank via a -I matmul, Act
     applies Sin.
  MM2 (TensorE, bf16, transposed): out[b,m] accumulated as 8 tiny
      [128b x 8m] matmuls per tile (lhsT = g columns, rhs = Wp tile);
      cost scales with the 8-wide moving dim instead of the 1024 batch.

PSUM: d-tiles 2 bufs x 2 banks, helper halves 3 bufs x 1 bank, acc 1 bank.

Scheduling: software-pipelined emission (subtract+Sin lag 2 tiles, MM2 lag
5) with ready-work-first ordering per engine stream; a PE "prewarm" train
of tiny matmuls keeps the tensor engine's p-state ramp warm so real
matmuls run at full clock; a leading dummy Sin pins the trig activation
table once.

Head (head3): HWDGE DMA issues serialize at ~625ns each and every
DMA-dependent start pays a 900ns semaphore-propagation penalty, so the
inputs ship as ONE combined tensor [wft tiles 0-1 | xt | wft tiles 2-31]
whose first chunk (xt half 0 + the first two weight slivers) is all tiles
0/1 need; those two tiles then run at [128,512] half granularity so the
first DVE op starts ~3.9us (vs 4.5us), the earliest the DMA chain allows.
Helper tiles (5,7,...,30) and knobs hill-climbed against TimelineSim.
"""
import sys

if "/opt/trn_rl_repo" not in sys.path:
    sys.path.insert(0, "/opt/trn_rl_repo")

import ml_dtypes
import numpy as np

import concourse.bacc as bacc
import concourse.mybir as mybir
import concourse.tile as tile
from concourse import bass_utils
from concourse import dve_ops
from concourse.dve_spec import C0, C1, C2, Spec, Src0, lower
from concourse.dve_uop import DveOpSpec

# Problem shape (hardcoded per contest contract).
B = 8192
D = 64
DA = D + 1  # contraction augmented with a ones-row (bias)
N = 4096
M = 8
NCORES = 8
BS = B // NCORES  # 1024 batch rows per core
P = 128
NT = N // P  # 32 feature tiles
FREE = 512  # MM1 matmul moving free dim (one PSUM bank)
NBG = BS // P  # 8 batch groups for the transposed MM2

f32 = mybir.dt.float32
f32r = mybir.dt.float32r
f16 = mybir.dt.float16
bf16 = mybir.dt.bfloat16

MAGIC = float(1.5 * 2**23)  # fp32 round-to-nearest-int magic constant
# sin(pi z) ~= k z (A - z^2)(B - z^2) minimax fit on [-1,1]; substituting
# z = 2r gives sin(2pi r) ~= KP * r (AQ - r^2)(BQ - r^2) on r in [-.5,.5].
AQ = 0.2512187245830011
BQ = 0.4600290215280054
KP = 53.520624390078666
# One fp32 ulp below 2*pi so |r|<=0.5 keeps Sin's argument inside [-pi,pi].
SCALE_SIN = float(np.nextafter(np.float32(2 * np.pi), np.float32(0)))

ND, NH = 23, 9  # DVE-path tiles vs Act+Pool+PE helper tiles


HELPER_TILES = (5, 7, 10, 13, 16, 19, 22, 25, 30)  # tuned by search


def _make_paths(nd=ND, nh=NH):
    """Helper-tile placement: denser early (while the DVE pipeline ramps),
    spacing 3 later; first/last tiles stay on the short DVE path.  The
    exact positions were hill-climbed against TimelineSim."""
    assert nd + nh == NT and nh == len(HELPER_TILES)
    out = ["d"] * NT
    for p in HELPER_TILES:
        out[p] = "h"
    return out


def _make_sine_op():
    """Custom DVE op: r = t - rne(t) (C0 = 1.5*2^23), out = r(C1-r^2)(C2-r^2).

    Registered into concourse.dve_ops at import time (idempotent)."""
    name = "SINE_QUINTIC_ANT"
    for o in dve_ops.OPS:
        if o.name == name:
            return o
    t = Src0
    u0 = t + C0
    u1 = u0 - C0
    r = t - u1
    s = r * r
    body = (r * (C1 - s)) * (C2 - s)

    def ref(in0, in1, s0, s1, imm2):
        tt = in0.astype(np.float32)
        u1 = ((tt + np.float32(s0)) - np.float32(s0)).astype(np.float32)
        r = (tt - u1).astype(np.float32)
        s = (r * r).astype(np.float32)
        return (
            (r * (np.float32(s1) - s)).astype(np.float32) * (np.float32(imm2) - s)
        ).astype(np.float32)

    spec = Spec(body=body, reference=ref)
    opcode = dve_ops._CUSTOM_DVE_ROW_BASE + len(dve_ops.OPS)
    dve_ops._SUB_OPCODE_FOR_NAME[name] = opcode
    shas = {}
    for ver in ("v3", "v4"):
        tmp = DveOpSpec(
            name=name, opcode=opcode, uops=lower(spec, ver=ver), rd1_en=False
        )
        shas[ver] = tmp.sha(ver)
    op = dve_ops.DveOp(name, spec, subdim=False, uops_sha=shas)
    dve_ops.OPS.append(op)
    dve_ops.CUSTOM_DVE_SPECS[name] = spec
    return op


def build(paths=None, prewarm=47, order=0, lag_s2=2, lag_mm2=5, gbufs=7, hwide=False, fphbufs=3, u0bufs=20, ubufs=20, dve_u=(), head_opt=0, interleave_s2=0, split_copy=0, head2=0, head3=0, wave_order=1, hhalf=1, dprio=0, mm2prio=0, tail_half=0):
    paths = paths or _make_paths()
    sine_op = _make_sine_op()
    Ident = mybir.ActivationFunctionType.Identity
    Sin = mybir.ActivationFunctionType.Sin
    nc = bacc.Bacc("TRN2", target_bir_lowering=False, debug=False, num_devices=NCORES)

    if head3:
        # One combined input tensor [wft tiles 0-1 | xt | wft tiles 2-31] so
        # the FIRST DMA (HWDGE issues serialize at ~625ns each) carries
        # everything tiles 0/1 need.
        xtw_d = nc.dram_tensor("xtw", [DA, 2 * P + BS + (NT - 2) * P], f32,
                               kind="ExternalInput").ap()
    else:
        xt_d = nc.dram_tensor("xt", [DA, BS], f32, kind="ExternalInput").ap()
        wft_d = nc.dram_tensor("wft", [DA, N], f32, kind="ExternalInput").ap()
    consts_d = nc.dram_tensor("consts", [P, P + NT * M], bf16, kind="ExternalInput").ap()
    out_d = nc.dram_tensor("out", [P, NBG, M], f32, kind="ExternalOutput").ap()

    with tile.TileContext(nc) as tc:
        with (
            tc.tile_pool(name="singles", bufs=1) as singles,
            tc.tile_pool(name="gpool", bufs=gbufs) as gpool,
            tc.tile_pool(name="u0pool", bufs=u0bufs) as u0pool,
            tc.tile_pool(name="upool", bufs=ubufs) as upool,
            tc.tile_pool(name="fpd", bufs=2, space="PSUM") as fpd,
            tc.tile_pool(name="fph", bufs=fphbufs, space="PSUM") as fph,
            tc.tile_pool(name="accpool", bufs=1, space="PSUM") as accpool,
        ):
            # Preload: few large DMAs split across the SP/Act hardware DGEs
            # and the Pool software DGE, ordered so the first MM1's inputs
            # (wft chunk 0, xt) land earliest.
            if head3:
                XW = 2 * P + BS  # start of wft tiles 2..31 in xtw
                xtw_sb = singles.tile([DA, 2 * P + BS + (NT - 2) * P], f32r,
                                      tag="xtw_sb")
            else:
                xt_sb = singles.tile([DA, BS], f32r, tag="xt_sb")
                wft_sb = singles.tile([DA, N], f32r, tag="wft_sb")
            consts_sb = singles.tile([P, P + NT * M], bf16, tag="consts")
            sinkp_sb = singles.tile([P, 1], f32, tag="sinkp")
            # dummy Sin as the first Act instruction pins the trig act-func
            # table once; Identity/Copy/memset_zero live in the same set.
            # Input is the framework's preamble-initialized const-zero AP.
            nc.scalar.activation(
                sinkp_sb[:], nc.const_aps.tensor(0.0, (P, 1)), Sin, scale=SCALE_SIN
            )
            if head3:
                # slivers01+xt0 | xt1 | consts (SWDGE, parallel issue) |
                # wft 2-7 | wft 8-19 | wft 20-31
                nc.sync.dma_start(xtw_sb[:, :2 * P + FREE],
                                  xtw_d[:, :2 * P + FREE].bitcast(f32r))
                nc.sync.dma_start(xtw_sb[:, 2 * P + FREE:XW],
                                  xtw_d[:, 2 * P + FREE:XW].bitcast(f32r))
                nc.gpsimd.dma_start(consts_sb, consts_d)
                nc.sync.dma_start(xtw_sb[:, XW:XW + 6 * P],
                                  xtw_d[:, XW:XW + 6 * P].bitcast(f32r))
                nc.sync.dma_start(xtw_sb[:, XW + 6 * P:XW + 18 * P],
                                  xtw_d[:, XW + 6 * P:XW + 18 * P].bitcast(f32r))
                nc.sync.dma_start(xtw_sb[:, XW + 18 * P:],
                                  xtw_d[:, XW + 18 * P:].bitcast(f32r))
            elif head2:
                # xt chunk 0 first (rhs of every tile's first MM1), then the
                # wft sliver for the first tiles, then the rest; first two
                # D-tiles run at half granularity so DVE starts ~1.4us sooner.
                nc.sync.dma_start(xt_sb[:, :FREE], xt_d[:, :FREE].bitcast(f32r))
                nc.scalar.dma_start(wft_sb[:, :FREE], wft_d[:, :FREE].bitcast(f32r))
                nc.sync.dma_start(xt_sb[:, FREE:], xt_d[:, FREE:].bitcast(f32r))
                nc.scalar.dma_start(consts_sb, consts_d)
                nc.sync.dma_start(
                    wft_sb[:, FREE:4 * FREE], wft_d[:, FREE:4 * FREE].bitcast(f32r)
                )
                nc.sync.dma_start(wft_sb[:, 4 * FREE:], wft_d[:, 4 * FREE:].bitcast(f32r))
            elif head_opt:
                # tile 0 needs only a 128-col sliver of wft; land it first
                nc.gpsimd.dma_start(xt_sb[:, :FREE], xt_d[:, :FREE].bitcast(f32r))
                nc.sync.dma_start(wft_sb[:, :P], wft_d[:, :P].bitcast(f32r))
                nc.sync.dma_start(xt_sb[:, FREE:], xt_d[:, FREE:].bitcast(f32r))
                nc.scalar.dma_start(consts_sb, consts_d)
                nc.sync.dma_start(wft_sb[:, P:FREE], wft_d[:, P:FREE].bitcast(f32r))
                nc.sync.dma_start(
                    wft_sb[:, FREE:4 * FREE], wft_d[:, FREE:4 * FREE].bitcast(f32r)
                )
                nc.sync.dma_start(wft_sb[:, 4 * FREE:], wft_d[:, 4 * FREE:].bitcast(f32r))
            else:
                nc.sync.dma_start(wft_sb[:, :FREE], wft_d[:, :FREE].bitcast(f32r))
                nc.gpsimd.dma_start(xt_sb, xt_d[:].bitcast(f32r))
                nc.scalar.dma_start(consts_sb, consts_d)
                nc.sync.dma_start(
                    wft_sb[:, FREE:4 * FREE], wft_d[:, FREE:4 * FREE].bitcast(f32r)
                )
                nc.sync.dma_start(
                    wft_sb[:, 4 * FREE:], wft_d[:, 4 * FREE:].bitcast(f32r)
                )
            if head3:
                xt_tiles = [xtw_sb[:, 2 * P + j * FREE:2 * P + (j + 1) * FREE]
                            for j in range(2)]

                def wsl(t):
                    if t < 2:
                        return xtw_sb[:, t * P:(t + 1) * P]
                    return xtw_sb[:, XW + (t - 2) * P:XW + (t - 1) * P]
            else:
                xt_tiles = [xt_sb[:, j * FREE:(j + 1) * FREE] for j in range(2)]
                wft_tiles = [wft_sb[:, c * FREE:(c + 1) * FREE] for c in range(8)]

                def wsl(t):
                    return wft_tiles[t // 4][:, (t % 4) * P:(t % 4 + 1) * P]
            negi_sb = consts_sb[:, :P]
            wps_sb = consts_sb[:, P:]
            magic_sb = singles.tile([P, 1], f32, tag="magicsb")
            nc.gpsimd.memset(magic_sb[:], MAGIC)
            # PE prewarm: a train of tiny matmuls on memset zeros keeps the PE
            # busy from t~0.4us, so the first real MM1s are decoded in the
            # MID p-state (and later ones at full speed) instead of LOW.
            warm_sb = singles.tile([P, 32], bf16, tag="warmsb")
            nc.gpsimd.memset(warm_sb[:], 0.0)

            acc = accpool.tile([P, NBG * M], f32)
            for _ in range(prewarm):
                nc.tensor.matmul(
                    acc[:32, :32], lhsT=warm_sb[:], rhs=warm_sb[:],
                    start=True, stop=True, skip_group_check=True,
                )

            fps_by_t = {}
            g_by_t = {}
            u0_by_t = {}

            from contextlib import nullcontext

            def emit_mm1(t):
                lhsT = wsl(t)
                if paths[t] == "d":
                    fps = fpd.tile([P, BS], f32)
                    fps_by_t[t] = fps
                    with tc.high_priority(offset=dprio) if dprio else nullcontext():
                        for j in range(2):
                            nc.tensor.matmul(
                                fps[:, j * FREE:(j + 1) * FREE],
                                lhsT=lhsT,
                                rhs=xt_tiles[j][:],
                                start=True,
                                stop=True,
                            )
                elif hwide:
                    fps = fph.tile([P, BS], f32)
                    fps_by_t[t] = fps
                    for j in range(2):
                        nc.tensor.matmul(
                            fps[:, j * FREE:(j + 1) * FREE],
                            lhsT=lhsT,
                            rhs=xt_tiles[j][:],
                            start=True,
                            stop=False,
                        )
                else:
                    halves = []
                    for j in range(2):
                        fh = fph.tile([P, FREE], f32)
                        halves.append(fh)
                        nc.tensor.matmul(
                            fh[:],
                            lhsT=lhsT,
                            rhs=xt_tiles[j][:],
                            start=True,
                            stop=False,
                        )
                    fps_by_t[t] = halves

            def emit_stage1(t):
                # produce either g (path d) or u0 halves (path h) from psum
                if paths[t] == "d":
                    g = gpool.tile([P, BS], bf16)
                    g_by_t[t] = g
                    if tail_half and t == NT - 1:
                        # last tile in halves: MM2s for batch-half 0 overlap
                        # the second half-op (subtile deps), shortening the
                        # post-DVE tail chain
                        for j in range(2):
                            sl = slice(j * FREE, (j + 1) * FREE)
                            nc.vector._custom_dve(
                                sine_op, out=g[:, sl], in0=fps_by_t[t][:, sl],
                                s0=MAGIC, s1=AQ, imm2=BQ
                            )
                    elif head_opt and t < head_opt:
                        for j in range(2):
                            sl = slice(j * FREE, (j + 1) * FREE)
                            nc.vector._custom_dve(
                                sine_op, out=g[:, sl], in0=fps_by_t[t][:, sl],
                                s0=MAGIC, s1=AQ, imm2=BQ
                            )
                    else:
                        nc.vector._custom_dve(
                            sine_op, out=g[:], in0=fps_by_t[t][:],
                            s0=MAGIC, s1=AQ, imm2=BQ
                        )
                elif hwide:
                    u0 = u0pool.tile([P, BS], f32)
                    nc.scalar.activation(
                        u0[:], fps_by_t[t][:], Ident, bias=magic_sb[:]
                    )
                    u = upool.tile([P, BS], bf16)
                    nc.gpsimd.tensor_scalar(
                        out=u[:], in0=u0[:], scalar1=MAGIC, scalar2=None,
                        op0=mybir.AluOpType.subtract,
                    )
                    u0_by_t[t] = u
                elif t in dve_u:
                    # DVE computes u = (t+magic)-magic directly from PSUM in
                    # one pass, relieving Act+Pool for this tile
                    us = []
                    for j in range(2):
                        u = upool.tile([P, FREE], bf16)
                        us.append(u)
                        nc.vector.tensor_scalar(
                            out=u[:], in0=fps_by_t[t][j][:],
                            scalar1=MAGIC, scalar2=MAGIC,
                            op0=mybir.AluOpType.add, op1=mybir.AluOpType.subtract,
                        )
                    u0_by_t[t] = us
                else:
                    us = []
                    for j in range(2):
                        u0 = u0pool.tile([P, FREE], f32)
                        nc.scalar.activation(
                            u0[:], fps_by_t[t][j][:], Ident, bias=magic_sb[:]
                        )
                        u = upool.tile([P, FREE], bf16)
                        us.append(u)
                        nc.gpsimd.tensor_scalar(
                            out=u[:], in0=u0[:], scalar1=MAGIC, scalar2=None,
                            op0=mybir.AluOpType.subtract,
                        )
                    u0_by_t[t] = us

            def emit_stage2(t):
                # path h: PE subtract (t - rne(t) -> r in psum), then Act Sin
                if paths[t] == "d":
                    return
                g = gpool.tile([P, BS], bf16)
                g_by_t[t] = g
                if interleave_s2 and not hwide:
                    # per-half sub->sin interleave: Act starts each Sin as
                    # soon as its own half's subtract lands
                    for j in range(2):
                        nc.tensor.matmul(
                            fps_by_t[t][j][:],
                            lhsT=negi_sb[:],
                            rhs=u0_by_t[t][j][:],
                            start=False,
                            stop=True,
                        )
                        nc.scalar.activation(
                            g[:, j * FREE:(j + 1) * FREE],
                            fps_by_t[t][j][:],
                            Sin,
                            scale=SCALE_SIN,
                        )
                elif hwide:
                    fps = fps_by_t[t]
                    u = u0_by_t[t]
                    for j in range(2):
                        nc.tensor.matmul(
                            fps[:, j * FREE:(j + 1) * FREE],
                            lhsT=negi_sb[:],
                            rhs=u[:, j * FREE:(j + 1) * FREE],
                            start=False,
                            stop=True,
                        )
                    nc.scalar.activation(g[:], fps[:], Sin, scale=SCALE_SIN)
                else:
                    for j in range(2):
                        nc.tensor.matmul(
                            fps_by_t[t][j][:],
                            lhsT=negi_sb[:],
                            rhs=u0_by_t[t][j][:],
                            start=False,
                            stop=True,
                        )
                    for j in range(2):
                        nc.scalar.activation(
                            g[:, j * FREE:(j + 1) * FREE],
                            fps_by_t[t][j][:],
                            Sin,
                            scale=SCALE_SIN,
                        )

            def emit_mm2(t):
                g = g_by_t[t]
                for bg in range(NBG):
                    nc.tensor.matmul(
                        acc[:, bg * M:(bg + 1) * M],
                        lhsT=g[:, bg * P:(bg + 1) * P],
                        rhs=wps_sb[:, t * M:(t + 1) * M],
                        start=(t == 0 and bg == 0),
                        stop=(t == NT - 1 and bg == NBG - 1),
                        skip_group_check=True,
                    )

            # Software-pipelined emission: stage2 lags 2 tiles, mm2 lags 3.
            # `order` picks the within-iteration emission order of the PE
            # work (accs/subs/mm1) to trade off which wait blocks the stream.
            def emit_iter(t):
                steps = {
                    0: ("acc", "sub", "mm1"),
                    1: ("mm1", "sub", "acc"),
                    2: ("mm1", "acc", "sub"),
                    3: ("sub", "mm1", "acc"),
                    4: ("acc", "mm1d", "sub", "mm1h"),
                    5: ("mm1d", "acc", "sub", "mm1h"),
                }[order]
                for s in steps:
                    if s == "acc" and 0 <= t - lag_mm2 < NT:
                        emit_mm2(t - lag_mm2)
                    elif s == "sub" and 0 <= t - lag_s2 < NT:
                        emit_stage2(t - lag_s2)
                    elif s == "mm1" and t < NT:
                        emit_mm1(t)
                        emit_stage1(t)
                    elif s == "mm1d" and t < NT and paths[t] == "d":
                        emit_mm1(t)
                        emit_stage1(t)
                    elif s == "mm1h" and t < NT and paths[t] != "d":
                        emit_mm1(t)
                        emit_stage1(t)

            t0 = 0
            if (head2 or head3) and hhalf:
                # Half-granular head: tiles 0/1 (must be 'd') emitted as
                # [128,512] halves in two waves so the first DVE op runs as
                # soon as xt chunk 0 + the wft sliver land.  K=2 == fpd bufs;
                # larger K would block PE on a unit the second wave frees.
                assert paths[0] == "d" and paths[1] == "d"
                t0 = 2
                for t in (0, 1):
                    fps_by_t[t] = fpd.tile([P, BS], f32, name="fps", tag="fps")
                    g_by_t[t] = gpool.tile([P, BS], bf16, name="g", tag="g")
                hh = [(0, 0), (1, 0), (0, 1), (1, 1)] if wave_order == 0 else \
                     [(0, 0), (0, 1), (1, 0), (1, 1)]
                for t, j in hh:
                    sl = slice(j * FREE, (j + 1) * FREE)
                    nc.tensor.matmul(
                        fps_by_t[t][:, sl],
                        lhsT=wsl(t),
                        rhs=xt_tiles[j][:],
                        start=True,
                        stop=True,
                    )
                    nc.vector._custom_dve(
                        sine_op, out=g_by_t[t][:, sl], in0=fps_by_t[t][:, sl],
                        s0=MAGIC, s1=AQ, imm2=BQ
                    )
            for t in range(t0, NT + max(lag_s2, lag_mm2)):
                emit_iter(t)

            out_sb = singles.tile([P, NBG * M], f32, tag="outsb")
            if split_copy:
                half = NBG * M // 2
                nc.vector.tensor_copy(out=out_sb[:, :half], in_=acc[:, :half])
                nc.scalar.activation(
                    out_sb[:, half:], acc[:, half:],
                    mybir.ActivationFunctionType.Identity,
                )
            else:
                nc.vector.tensor_copy(out=out_sb[:], in_=acc[:])
            nc.sync.dma_start(out_d, out_sb[:])
    nc.compile()
    return nc


_NC = None
# Build knobs used by _get_nc()/run(); keep head3 here in sync with
# _prep_in_maps' input layout.  TimelineSim: 35844 ns (baseline 36440).
BUILD_KW = {"head3": 1, "gbufs": 8, "order": 3}


def _get_nc():
    global _NC
    if _NC is None:
        _NC = build(**BUILD_KW)
    return _NC


def _prep_in_maps(x, Wf, bf, Wp, paths=None):
    paths = paths or BUILD_KW.get("paths") or _make_paths()
    scale = np.float64(np.sqrt(2.0 / N))
    inv2pi = np.float64(1.0) / (2.0 * np.pi)
    # [65, 4096]: rows 0-63 = (Wf/2pi)^T, row 64 = bf/2pi + 1/4 (cos->sin)
    wft = np.empty((DA, N), dtype=np.float32)
    wft[:D] = (Wf.astype(np.float64) * inv2pi).astype(np.float32).T
    wft[D] = (bf.astype(np.float64) * inv2pi + 0.25).astype(np.float32)
    # Wp scaled per tile: DVE-path tiles additionally absorb the quintic's
    # leading coefficient KP.  [128, NT, M] in bf16.
    wps64 = Wp.astype(np.float64).reshape(NT, P, M) * scale
    for t in range(NT):
        if paths[t] == "d":
            wps64[t] *= KP
    consts = np.empty((P, P + NT * M), dtype=ml_dtypes.bfloat16)
    consts[:, :P] = (-np.eye(P)).astype(ml_dtypes.bfloat16)
    consts[:, P:] = (
        np.ascontiguousarray(wps64.transpose(1, 0, 2))
        .reshape(P, NT * M)
        .astype(ml_dtypes.bfloat16)
    )
    head3 = BUILD_KW.get("head3", 0)
    in_maps = []
    for c in range(NCORES):
        xt = np.empty((DA, BS), dtype=np.float32)
        xt[:D] = x[c * BS:(c + 1) * BS].astype(np.float32).T
        xt[D] = 1.0
        if head3:
            xtw = np.empty((DA, 2 * P + BS + (NT - 2) * P), dtype=np.float32)
            xtw[:, :2 * P] = wft[:, :2 * P]
            xtw[:, 2 * P:2 * P + BS] = xt
            xtw[:, 2 * P + BS:] = wft[:, 2 * P:]
            in_maps.append({"xtw": xtw, "consts": consts})
        else:
            in_maps.append({"xt": xt, "wft": wft, "consts": consts})
    return in_maps


def run(x, Wf, bf, Wp, trace=False):
    nc = _get_nc()
    in_maps = _prep_in_maps(x, Wf, bf, Wp)
    res = bass_utils.run_bass_kernel_spmd(
        nc, in_maps, core_ids=list(range(NCORES)), trace=trace
    )
    out = np.empty((B, M), dtype=np.float32)
    for c in range(NCORES):
        # device out: [P, NBG, M] with out[bg*128+p, m] = dev[p, bg, m]
        dev = res.results[c]["out"]
        out[c * BS:(c + 1) * BS, :] = dev.transpose(1, 0, 2).reshape(BS, M)
    return out, res


def kernel(x, Wf, bf, Wp):
    x = np.asarray(x)
    Wf = np.asarray(Wf)
    bf = np.asarray(bf)
    Wp = np.asarray(Wp)
    out, _ = run(x, Wf, bf, Wp, trace=False)
    return out



# revision 21
# speedup vs baseline: 1.0044x; 1.0044x over previous
"""Trainium2 Bass kernel for nn_KernelMachine (random-feature kernel machine).

Computes out = (sqrt(2/N) * cos(x @ Wf^T + bf)) @ Wp on 8 NeuronCores,
data-parallel over the batch dimension (1024 rows/core), no collectives.

Per-core pipeline, per 128-wide tile of the N=4096 feature dim, with the
elementwise work load-balanced across DVE and the Activation engine:

  MM1 (TensorE, f32r): t = [x | 1] @ [Wf/2pi | bf']  -> PSUM (in turns)
      (bias and the cos->sin quarter-turn fold into the ones-row weights)
  then one of two per-tile paths to g ~ sin(2pi t):
   - path 'd' (DVE): one fused custom op on [128,1024]: r = t - rne(t) via
     the 1.5*2^23 magic constant, then the factored quintic r(A-r^2)(B-r^2)
     which is sin(2pi r)/K for K=53.52...  (K folds into this tile's Wp)
   - path 'h' (Act+Pool+PE, two [128,512] halves): Act u0 = t + 1.5*2^23
     (= magic + rne(t), SBUF), Pool u = u0 - magic (bf16, small ints),
     PE accumulates t - u into the same PSUM bank via a -I matmul, Act
     applies Sin.
  MM2 (TensorE, bf16, transposed): out[b,m] accumulated as 8 tiny
      [128b x 8m] matmuls per tile (lhsT = g columns, rhs = Wp tile);
      cost scales with the 8-wide moving dim instead of the 1024 batch.

PSUM: d-tiles 2 bufs x 2 banks, helper halves 3 bufs x 1 bank, acc 1 bank.

Scheduling: software-pipelined emission (subtract+Sin lag 2 tiles, MM2 lag
5) with ready-work-first ordering per engine stream; a PE "prewarm" train
of tiny matmuls keeps the tensor engine's p-state ramp warm so real
matmuls run at full clock; a leading dummy Sin pins the trig activation
table once.

Head (head3): HWDGE DMA issues serialize at ~625ns each and every
DMA-dependent start pays a 900ns semaphore-propagation penalty, so the
inputs ship as ONE combined tensor [wft tiles 0-1 | xt | wft tiles 2-31]
whose first chunk (xt half 0 + the first two weight slivers) is all tiles
0/1 need; those two tiles then run at [128,512] half granularity so the
first DVE op starts ~3.9us (vs 4.5us), the earliest the DMA chain allows.
Helper tiles (5,7,...,30) and knobs hill-climbed against TimelineSim.
"""
import sys

if "/opt/trn_rl_repo" not in sys.path:
    sys.path.insert(0, "/opt/trn_rl_repo")

import ml_dtypes
import numpy as np

import concourse.bacc as bacc
import concourse.mybir as mybir
import concourse.tile as tile
from concourse import bass_utils
from concourse import dve_ops
from concourse.dve_spec import C0, C1, C2, Spec, Src0, lower
from concourse.dve_uop import DveOpSpec

# Problem shape (hardcoded per contest contract).
B = 8192
D = 64
DA = D + 1  # contraction augmented with a ones-row (bias)
N = 4096
M = 8
NCORES = 8
BS = B // NCORES  # 1024 batch rows per core
P = 128
NT = N // P  # 32 feature tiles
FREE = 512  # MM1 matmul moving free dim (one PSUM bank)
NBG = BS // P  # 8 batch groups for the transposed MM2

f32 = mybir.dt.float32
f32r = mybir.dt.float32r
f16 = mybir.dt.float16
bf16 = mybir.dt.bfloat16

MAGIC = float(1.5 * 2**23)  # fp32 round-to-nearest-int magic constant
# sin(pi z) ~= k z (A - z^2)(B - z^2) minimax fit on [-1,1]; substituting
# z = 2r gives sin(2pi r) ~= KP * r (AQ - r^2)(BQ - r^2) on r in [-.5,.5].
AQ = 0.2512187245830011
BQ = 0.4600290215280054
KP = 53.520624390078666
# One fp32 ulp below 2*pi so |r|<=0.5 keeps Sin's argument inside [-pi,pi].
SCALE_SIN = float(np.nextafter(np.float32(2 * np.pi), np.float32(0)))

ND, NH = 23, 9  # DVE-path tiles vs Act+Pool+PE helper tiles


HELPER_TILES = (5, 7, 10, 13, 16, 19, 22, 25, 30)  # tuned by search


def _make_paths(nd=ND, nh=NH):
    """Helper-tile placement: denser early (while the DVE pipeline ramps),
    spacing 3 later; first/last tiles stay on the short DVE path.  The
    exact positions were hill-climbed against TimelineSim."""
    assert nd + nh == NT and nh == len(HELPER_TILES)
    out = ["d"] * NT
    for p in HELPER_TILES:
        out[p] = "h"
    return out


def _make_sine_op():
    """Custom DVE op: r = t - rne(t) (C0 = 1.5*2^23), out = r(C1-r^2)(C2-r^2).

    Registered into concourse.dve_ops at import time (idempotent)."""
    name = "SINE_QUINTIC_ANT"
    for o in dve_ops.OPS:
        if o.name == name:
            return o
    t = Src0
    u0 = t + C0
    u1 = u0 - C0
    r = t - u1
    s = r * r
    body = (r * (C1 - s)) * (C2 - s)

    def ref(in0, in1, s0, s1, imm2):
        tt = in0.astype(np.float32)
        u1 = ((tt + np.float32(s0)) - np.float32(s0)).astype(np.float32)
        r = (tt - u1).astype(np.float32)
        s = (r * r).astype(np.float32)
        return (
            (r * (np.float32(s1) - s)).astype(np.float32) * (np.float32(imm2) - s)
        ).astype(np.float32)

    spec = Spec(body=body, reference=ref)
    opcode = dve_ops._CUSTOM_DVE_ROW_BASE + len(dve_ops.OPS)
    dve_ops._SUB_OPCODE_FOR_NAME[name] = opcode
    shas = {}
    for ver in ("v3", "v4"):
        tmp = DveOpSpec(
            name=name, opcode=opcode, uops=lower(spec, ver=ver), rd1_en=False
        )
        shas[ver] = tmp.sha(ver)
    op = dve_ops.DveOp(name, spec, subdim=False, uops_sha=shas)
    dve_ops.OPS.append(op)
    dve_ops.CUSTOM_DVE_SPECS[name] = spec
    return op


def build(paths=None, prewarm=47, order=0, lag_s2=2, lag_mm2=5, gbufs=7, hwide=False, fphbufs=3, u0bufs=20, ubufs=20, dve_u=(), head_opt=0, interleave_s2=0, split_copy=0, head2=0, head3=0, wave_order=1, hhalf=2, dprio=0, mm2prio=0, tail_half=0):
    paths = paths or _make_paths()
    sine_op = _make_sine_op()
    Ident = mybir.ActivationFunctionType.Identity
    Sin = mybir.ActivationFunctionType.Sin
    nc = bacc.Bacc("TRN2", target_bir_lowering=False, debug=False, num_devices=NCORES)

    if head3:
        # One combined input tensor [wft tiles 0-1 | xt | wft tiles 2-31] so
        # the FIRST DMA (HWDGE issues serialize at ~625ns each) carries
        # everything tiles 0/1 need.
        xtw_d = nc.dram_tensor("xtw", [DA, 2 * P + BS + (NT - 2) * P], f32,
                               kind="ExternalInput").ap()
    else:
        xt_d = nc.dram_tensor("xt", [DA, BS], f32, kind="ExternalInput").ap()
        wft_d = nc.dram_tensor("wft", [DA, N], f32, kind="ExternalInput").ap()
    consts_d = nc.dram_tensor("consts", [P, P + NT * M], bf16, kind="ExternalInput").ap()
    out_d = nc.dram_tensor("out", [P, NBG, M], f32, kind="ExternalOutput").ap()

    with tile.TileContext(nc) as tc:
        with (
            tc.tile_pool(name="singles", bufs=1) as singles,
            tc.tile_pool(name="gpool", bufs=gbufs) as gpool,
            tc.tile_pool(name="u0pool", bufs=u0bufs) as u0pool,
            tc.tile_pool(name="upool", bufs=ubufs) as upool,
            tc.tile_pool(name="fpd", bufs=2, space="PSUM") as fpd,
            tc.tile_pool(name="fph", bufs=fphbufs, space="PSUM") as fph,
            tc.tile_pool(name="accpool", bufs=1, space="PSUM") as accpool,
        ):
            # Preload: few large DMAs split across the SP/Act hardware DGEs
            # and the Pool software DGE, ordered so the first MM1's inputs
            # (wft chunk 0, xt) land earliest.
            if head3:
                XW = 2 * P + BS  # start of wft tiles 2..31 in xtw
                xtw_sb = singles.tile([DA, 2 * P + BS + (NT - 2) * P], f32r,
                                      tag="xtw_sb")
            else:
                xt_sb = singles.tile([DA, BS], f32r, tag="xt_sb")
                wft_sb = singles.tile([DA, N], f32r, tag="wft_sb")
            consts_sb = singles.tile([P, P + NT * M], bf16, tag="consts")
            sinkp_sb = singles.tile([P, 1], f32, tag="sinkp")
            # dummy Sin as the first Act instruction pins the trig act-func
            # table once; Identity/Copy/memset_zero live in the same set.
            # Input is the framework's preamble-initialized const-zero AP.
            nc.scalar.activation(
                sinkp_sb[:], nc.const_aps.tensor(0.0, (P, 1)), Sin, scale=SCALE_SIN
            )
            if head3:
                # slivers01+xt0 | xt1 | consts (SWDGE, parallel issue) |
                # wft 2-7 | wft 8-19 | wft 20-31
                nc.sync.dma_start(xtw_sb[:, :2 * P + FREE],
                                  xtw_d[:, :2 * P + FREE].bitcast(f32r))
                nc.sync.dma_start(xtw_sb[:, 2 * P + FREE:XW],
                                  xtw_d[:, 2 * P + FREE:XW].bitcast(f32r))
                nc.gpsimd.dma_start(consts_sb, consts_d)
                nc.sync.dma_start(xtw_sb[:, XW:XW + 6 * P],
                                  xtw_d[:, XW:XW + 6 * P].bitcast(f32r))
                nc.sync.dma_start(xtw_sb[:, XW + 6 * P:XW + 18 * P],
                                  xtw_d[:, XW + 6 * P:XW + 18 * P].bitcast(f32r))
                nc.sync.dma_start(xtw_sb[:, XW + 18 * P:],
                                  xtw_d[:, XW + 18 * P:].bitcast(f32r))
            elif head2:
                # xt chunk 0 first (rhs of every tile's first MM1), then the
                # wft sliver for the first tiles, then the rest; first two
                # D-tiles run at half granularity so DVE starts ~1.4us sooner.
                nc.sync.dma_start(xt_sb[:, :FREE], xt_d[:, :FREE].bitcast(f32r))
                nc.scalar.dma_start(wft_sb[:, :FREE], wft_d[:, :FREE].bitcast(f32r))
                nc.sync.dma_start(xt_sb[:, FREE:], xt_d[:, FREE:].bitcast(f32r))
                nc.scalar.dma_start(consts_sb, consts_d)
                nc.sync.dma_start(
                    wft_sb[:, FREE:4 * FREE], wft_d[:, FREE:4 * FREE].bitcast(f32r)
                )
                nc.sync.dma_start(wft_sb[:, 4 * FREE:], wft_d[:, 4 * FREE:].bitcast(f32r))
            elif head_opt:
                # tile 0 needs only a 128-col sliver of wft; land it first
                nc.gpsimd.dma_start(xt_sb[:, :FREE], xt_d[:, :FREE].bitcast(f32r))
                nc.sync.dma_start(wft_sb[:, :P], wft_d[:, :P].bitcast(f32r))
                nc.sync.dma_start(xt_sb[:, FREE:], xt_d[:, FREE:].bitcast(f32r))
                nc.scalar.dma_start(consts_sb, consts_d)
                nc.sync.dma_start(wft_sb[:, P:FREE], wft_d[:, P:FREE].bitcast(f32r))
                nc.sync.dma_start(
                    wft_sb[:, FREE:4 * FREE], wft_d[:, FREE:4 * FREE].bitcast(f32r)
                )
                nc.sync.dma_start(wft_sb[:, 4 * FREE:], wft_d[:, 4 * FREE:].bitcast(f32r))
            else:
                nc.sync.dma_start(wft_sb[:, :FREE], wft_d[:, :FREE].bitcast(f32r))
                nc.gpsimd.dma_start(xt_sb, xt_d[:].bitcast(f32r))
                nc.scalar.dma_start(consts_sb, consts_d)
                nc.sync.dma_start(
                    wft_sb[:, FREE:4 * FREE], wft_d[:, FREE:4 * FREE].bitcast(f32r)
                )
                nc.sync.dma_start(
                    wft_sb[:, 4 * FREE:], wft_d[:, 4 * FREE:].bitcast(f32r)
                )
            if head3:
                xt_tiles = [xtw_sb[:, 2 * P + j * FREE:2 * P + (j + 1) * FREE]
                            for j in range(2)]

                def wsl(t):
                    if t < 2:
                        return xtw_sb[:, t * P:(t + 1) * P]
                    return xtw_sb[:, XW + (t - 2) * P:XW + (t - 1) * P]
            else:
                xt_tiles = [xt_sb[:, j * FREE:(j + 1) * FREE] for j in range(2)]
                wft_tiles = [wft_sb[:, c * FREE:(c + 1) * FREE] for c in range(8)]

                def wsl(t):
                    return wft_tiles[t // 4][:, (t % 4) * P:(t % 4 + 1) * P]
            negi_sb = consts_sb[:, :P]
            wps_sb = consts_sb[:, P:]
            magic_sb = singles.tile([P, 1], f32, tag="magicsb")
            nc.gpsimd.memset(magic_sb[:], MAGIC)
            # PE prewarm: a train of tiny matmuls on memset zeros keeps the PE
            # busy from t~0.4us, so the first real MM1s are decoded in the
            # MID p-state (and later ones at full speed) instead of LOW.
            warm_sb = singles.tile([P, 32], bf16, tag="warmsb")
            nc.gpsimd.memset(warm_sb[:], 0.0)

            acc = accpool.tile([P, NBG * M], f32)
            for _ in range(prewarm):
                nc.tensor.matmul(
                    acc[:32, :32], lhsT=warm_sb[:], rhs=warm_sb[:],
                    start=True, stop=True, skip_group_check=True,
                )

            fps_by_t = {}
            g_by_t = {}
            u0_by_t = {}

            from contextlib import nullcontext

            def emit_mm1(t):
                lhsT = wsl(t)
                if paths[t] == "d":
                    fps = fpd.tile([P, BS], f32)
                    fps_by_t[t] = fps
                    with tc.high_priority(offset=dprio) if dprio else nullcontext():
                        for j in range(2):
                            nc.tensor.matmul(
                                fps[:, j * FREE:(j + 1) * FREE],
                                lhsT=lhsT,
                                rhs=xt_tiles[j][:],
                                start=True,
                                stop=True,
                            )
                elif hwide:
                    fps = fph.tile([P, BS], f32)
                    fps_by_t[t] = fps
                    for j in range(2):
                        nc.tensor.matmul(
                            fps[:, j * FREE:(j + 1) * FREE],
                            lhsT=lhsT,
                            rhs=xt_tiles[j][:],
                            start=True,
                            stop=False,
                        )
                else:
                    halves = []
                    for j in range(2):
                        fh = fph.tile([P, FREE], f32)
                        halves.append(fh)
                        nc.tensor.matmul(
                            fh[:],
                            lhsT=lhsT,
                            rhs=xt_tiles[j][:],
                            start=True,
                            stop=False,
                        )
                    fps_by_t[t] = halves

            def emit_stage1(t):
                # produce either g (path d) or u0 halves (path h) from psum
                if paths[t] == "d":
                    g = gpool.tile([P, BS], bf16)
                    g_by_t[t] = g
                    if tail_half and t == NT - 1:
                        # last tile in halves: MM2s for batch-half 0 overlap
                        # the second half-op (subtile deps), shortening the
                        # post-DVE tail chain
                        for j in range(2):
                            sl = slice(j * FREE, (j + 1) * FREE)
                            nc.vector._custom_dve(
                                sine_op, out=g[:, sl], in0=fps_by_t[t][:, sl],
                                s0=MAGIC, s1=AQ, imm2=BQ
                            )
                    elif head_opt and t < head_opt:
                        for j in range(2):
                            sl = slice(j * FREE, (j + 1) * FREE)
                            nc.vector._custom_dve(
                                sine_op, out=g[:, sl], in0=fps_by_t[t][:, sl],
                                s0=MAGIC, s1=AQ, imm2=BQ
                            )
                    else:
                        nc.vector._custom_dve(
                            sine_op, out=g[:], in0=fps_by_t[t][:],
                            s0=MAGIC, s1=AQ, imm2=BQ
                        )
                elif hwide:
                    u0 = u0pool.tile([P, BS], f32)
                    nc.scalar.activation(
                        u0[:], fps_by_t[t][:], Ident, bias=magic_sb[:]
                    )
                    u = upool.tile([P, BS], bf16)
                    nc.gpsimd.tensor_scalar(
                        out=u[:], in0=u0[:], scalar1=MAGIC, scalar2=None,
                        op0=mybir.AluOpType.subtract,
                    )
                    u0_by_t[t] = u
                elif t in dve_u:
                    # DVE computes u = (t+magic)-magic directly from PSUM in
                    # one pass, relieving Act+Pool for this tile
                    us = []
                    for j in range(2):
                        u = upool.tile([P, FREE], bf16)
                        us.append(u)
                        nc.vector.tensor_scalar(
                            out=u[:], in0=fps_by_t[t][j][:],
                            scalar1=MAGIC, scalar2=MAGIC,
                            op0=mybir.AluOpType.add, op1=mybir.AluOpType.subtract,
                        )
                    u0_by_t[t] = us
                else:
                    us = []
                    for j in range(2):
                        u0 = u0pool.tile([P, FREE], f32)
                        nc.scalar.activation(
                            u0[:], fps_by_t[t][j][:], Ident, bias=magic_sb[:]
                        )
                        u = upool.tile([P, FREE], bf16)
                        us.append(u)
                        nc.gpsimd.tensor_scalar(
                            out=u[:], in0=u0[:], scalar1=MAGIC, scalar2=None,
                            op0=mybir.AluOpType.subtract,
                        )
                    u0_by_t[t] = us

            def emit_stage2(t):
                # path h: PE subtract (t - rne(t) -> r in psum), then Act Sin
                if paths[t] == "d":
                    return
                g = gpool.tile([P, BS], bf16)
                g_by_t[t] = g
                if interleave_s2 and not hwide:
                    # per-half sub->sin interleave: Act starts each Sin as
                    # soon as its own half's subtract lands
                    for j in range(2):
                        nc.tensor.matmul(
                            fps_by_t[t][j][:],
                            lhsT=negi_sb[:],
                            rhs=u0_by_t[t][j][:],
                            start=False,
                            stop=True,
                        )
                        nc.scalar.activation(
                            g[:, j * FREE:(j + 1) * FREE],
                            fps_by_t[t][j][:],
                            Sin,
                            scale=SCALE_SIN,
                        )
                elif hwide:
                    fps = fps_by_t[t]
                    u = u0_by_t[t]
                    for j in range(2):
                        nc.tensor.matmul(
                            fps[:, j * FREE:(j + 1) * FREE],
                            lhsT=negi_sb[:],
                            rhs=u[:, j * FREE:(j + 1) * FREE],
                            start=False,
                            stop=True,
                        )
                    nc.scalar.activation(g[:], fps[:], Sin, scale=SCALE_SIN)
                else:
                    for j in range(2):
                        nc.tensor.matmul(
                            fps_by_t[t][j][:],
                            lhsT=negi_sb[:],
                            rhs=u0_by_t[t][j][:],
                            start=False,
                            stop=True,
                        )
                    for j in range(2):
                        nc.scalar.activation(
                            g[:, j * FREE:(j + 1) * FREE],
                            fps_by_t[t][j][:],
                            Sin,
                            scale=SCALE_SIN,
                        )

            def emit_mm2(t):
                g = g_by_t[t]
                for bg in range(NBG):
                    nc.tensor.matmul(
                        acc[:, bg * M:(bg + 1) * M],
                        lhsT=g[:, bg * P:(bg + 1) * P],
                        rhs=wps_sb[:, t * M:(t + 1) * M],
                        start=(t == 0 and bg == 0),
                        stop=(t == NT - 1 and bg == NBG - 1),
                        skip_group_check=True,
                    )

            # Software-pipelined emission: stage2 lags 2 tiles, mm2 lags 3.
            # `order` picks the within-iteration emission order of the PE
            # work (accs/subs/mm1) to trade off which wait blocks the stream.
            def emit_iter(t):
                steps = {
                    0: ("acc", "sub", "mm1"),
                    1: ("mm1", "sub", "acc"),
                    2: ("mm1", "acc", "sub"),
                    3: ("sub", "mm1", "acc"),
                    4: ("acc", "mm1d", "sub", "mm1h"),
                    5: ("mm1d", "acc", "sub", "mm1h"),
                }[order]
                for s in steps:
                    if s == "acc" and 0 <= t - lag_mm2 < NT:
                        emit_mm2(t - lag_mm2)
                    elif s == "sub" and 0 <= t - lag_s2 < NT:
                        emit_stage2(t - lag_s2)
                    elif s == "mm1" and t < NT:
                        emit_mm1(t)
                        emit_stage1(t)
                    elif s == "mm1d" and t < NT and paths[t] == "d":
                        emit_mm1(t)
                        emit_stage1(t)
                    elif s == "mm1h" and t < NT and paths[t] != "d":
                        emit_mm1(t)
                        emit_stage1(t)

            t0 = 0
            if (head2 or head3) and hhalf:
                nh2 = min(int(hhalf), 2)
                # Half-granular head: tiles 0/1 (must be 'd') emitted as
                # [128,512] halves in two waves so the first DVE op runs as
                # soon as xt chunk 0 + the wft sliver land.  K=2 == fpd bufs;
                # larger K would block PE on a unit the second wave frees.
                assert all(paths[i] == "d" for i in range(nh2))
                t0 = nh2
                for t in range(nh2):
                    fps_by_t[t] = fpd.tile([P, BS], f32, name="fps", tag="fps")
                    g_by_t[t] = gpool.tile([P, BS], bf16, name="g", tag="g")
                if wave_order == 0:
                    hh = [(t, j) for j in (0, 1) for t in range(nh2)]
                else:
                    hh = [(t, j) for t in range(nh2) for j in (0, 1)]
                for t, j in hh:
                    sl = slice(j * FREE, (j + 1) * FREE)
                    nc.tensor.matmul(
                        fps_by_t[t][:, sl],
                        lhsT=wsl(t),
                        rhs=xt_tiles[j][:],
                        start=True,
                        stop=True,
                    )
                    nc.vector._custom_dve(
                        sine_op, out=g_by_t[t][:, sl], in0=fps_by_t[t][:, sl],
                        s0=MAGIC, s1=AQ, imm2=BQ
                    )
            for t in range(t0, NT + max(lag_s2, lag_mm2)):
                emit_iter(t)

            out_sb = singles.tile([P, NBG * M], f32, tag="outsb")
            if split_copy:
                half = NBG * M // 2
                nc.vector.tensor_copy(out=out_sb[:, :half], in_=acc[:, :half])
                nc.scalar.activation(
                    out_sb[:, half:], acc[:, half:],
                    mybir.ActivationFunctionType.Identity,
                )
            else:
                nc.vector.tensor_copy(out=out_sb[:], in_=acc[:])
            nc.sync.dma_start(out_d, out_sb[:])
    nc.compile()
    return nc


_NC = None
# Build knobs used by _get_nc()/run(); keep head3 here in sync with
# _prep_in_maps' input layout.  TimelineSim: 35688 ns (baseline 36440).
BUILD_KW = {"head3": 1, "gbufs": 8, "order": 3, "hhalf": 1}


def _get_nc():
    global _NC
    if _NC is None:
        _NC = build(**BUILD_KW)
    return _NC


def _prep_in_maps(x, Wf, bf, Wp, paths=None):
    paths = paths or BUILD_KW.get("paths") or _make_paths()
    scale = np.float64(np.sqrt(2.0 / N))
    inv2pi = np.float64(1.0) / (2.0 * np.pi)
    # [65, 4096]: rows 0-63 = (Wf/2pi)^T, row 64 = bf/2pi + 1/4 (cos->sin)
    wft = np.empty((DA, N), dtype=np.float32)
    wft[:D] = (Wf.astype(np.float64) * inv2pi).astype(np.float32).T
    wft[D] = (bf.astype(np.float64) * inv2pi + 0.25).astype(np.float32)
    # Wp scaled per tile: DVE-path tiles additionally absorb the quintic's
    # leading coefficient KP.  [128, NT, M] in bf16.
    wps64 = Wp.astype(np.float64).reshape(NT, P, M) * scale
    for t in range(NT):
        if paths[t] == "d":
            wps64[t] *= KP
    consts = np.empty((P, P + NT * M), dtype=ml_dtypes.bfloat16)
    consts[:, :P] = (-np.eye(P)).astype(ml_dtypes.bfloat16)
    consts[:, P:] = (
        np.ascontiguousarray(wps64.transpose(1, 0, 2))
        .reshape(P, NT * M)
        .astype(ml_dtypes.bfloat16)
    )
    head3 = BUILD_KW.get("head3", 0)
    in_maps = []
    for c in range(NCORES):
        xt = np.empty((DA, BS), dtype=np.float32)
        xt[:D] = x[c * BS:(c + 1) * BS].astype(np.float32).T
        xt[D] = 1.0
        if head3:
            xtw = np.empty((DA, 2 * P + BS + (NT - 2) * P), dtype=np.float32)
            xtw[:, :2 * P] = wft[:, :2 * P]
            xtw[:, 2 * P:2 * P + BS] = xt
            xtw[:, 2 * P + BS:] = wft[:, 2 * P:]
            in_maps.append({"xtw": xtw, "consts": consts})
        else:
            in_maps.append({"xt": xt, "wft": wft, "consts": consts})
    return in_maps


def run(x, Wf, bf, Wp, trace=False):
    nc = _get_nc()
    in_maps = _prep_in_maps(x, Wf, bf, Wp)
    res = bass_utils.run_bass_kernel_spmd(
        nc, in_maps, core_ids=list(range(NCORES)), trace=trace
    )
    out = np.empty((B, M), dtype=np.float32)
    for c in range(NCORES):
        # device out: [P, NBG, M] with out[bg*128+p, m] = dev[p, bg, m]
        dev = res.results[c]["out"]
        out[c * BS:(c + 1) * BS, :] = dev.transpose(1, 0, 2).reshape(BS, M)
    return out, res


def kernel(x, Wf, bf, Wp):
    x = np.asarray(x)
    Wf = np.asarray(Wf)
    bf = np.asarray(bf)
    Wp = np.asarray(Wp)
    out, _ = run(x, Wf, bf, Wp, trace=False)
    return out



# revision 23
# speedup vs baseline: 1.0050x; 1.0006x over previous
"""Trainium2 Bass kernel for nn_KernelMachine (random-feature kernel machine).

Computes out = (sqrt(2/N) * cos(x @ Wf^T + bf)) @ Wp on 8 NeuronCores,
data-parallel over the batch dimension (1024 rows/core), no collectives.

Per-core pipeline, per 128-wide tile of the N=4096 feature dim, with the
elementwise work load-balanced across DVE and the Activation engine:

  MM1 (TensorE, f32r): t = [x | 1] @ [Wf/2pi | bf']  -> PSUM (in turns)
      (bias and the cos->sin quarter-turn fold into the ones-row weights)
  then one of two per-tile paths to g ~ sin(2pi t):
   - path 'd' (DVE): one fused custom op on [128,1024]: r = t - rne(t) via
     the 1.5*2^23 magic constant, then the factored quintic r(A-r^2)(B-r^2)
     which is sin(2pi r)/K for K=53.52...  (K folds into this tile's Wp)
   - path 'h' (Act+Pool+PE, two [128,512] halves): Act u0 = t + 1.5*2^23
     (= magic + rne(t), SBUF), Pool u = u0 - magic (bf16, small ints),
     PE accumulates t - u into the same PSUM bank via a -I matmul, Act
     applies Sin.
  MM2 (TensorE, bf16, transposed): out[b,m] accumulated as 8 tiny
      [128b x 8m] matmuls per tile (lhsT = g columns, rhs = Wp tile);
      cost scales with the 8-wide moving dim instead of the 1024 batch.

PSUM: d-tiles 2 bufs x 2 banks, helper halves 3 bufs x 1 bank, acc 1 bank.

Scheduling: software-pipelined emission (subtract+Sin lag 2 tiles, MM2 lag
5) with ready-work-first ordering per engine stream; a PE "prewarm" train
of tiny matmuls keeps the tensor engine's p-state ramp warm so real
matmuls run at full clock; a leading dummy Sin pins the trig activation
table once.

Head (head3): HWDGE DMA issues serialize at ~625ns each and every
DMA-dependent start pays a 900ns semaphore-propagation penalty, so the
inputs ship as ONE combined tensor [wft tiles 0-1 | xt | wft tiles 2-31]
whose first chunk (xt half 0 + the first two weight slivers) is all tiles
0/1 need; tile 0 then runs at [128,512] half granularity so the first
DVE op starts ~3.9us (vs 4.5us), the earliest the DMA chain allows.
Helper tiles (5,7,...,30) and knobs hill-climbed against TimelineSim.
"""
import sys

if "/opt/trn_rl_repo" not in sys.path:
    sys.path.insert(0, "/opt/trn_rl_repo")

import ml_dtypes
import numpy as np

import concourse.bacc as bacc
import concourse.mybir as mybir
import concourse.tile as tile
from concourse import bass_utils
from concourse import dve_ops
from concourse.dve_spec import C0, C1, C2, Spec, Src0, lower
from concourse.dve_uop import DveOpSpec

# Problem shape (hardcoded per contest contract).
B = 8192
D = 64
DA = D + 1  # contraction augmented with a ones-row (bias)
N = 4096
M = 8
NCORES = 8
BS = B // NCORES  # 1024 batch rows per core
P = 128
NT = N // P  # 32 feature tiles
FREE = 512  # MM1 matmul moving free dim (one PSUM bank)
NBG = BS // P  # 8 batch groups for the transposed MM2

f32 = mybir.dt.float32
f32r = mybir.dt.float32r
f16 = mybir.dt.float16
bf16 = mybir.dt.bfloat16

MAGIC = float(1.5 * 2**23)  # fp32 round-to-nearest-int magic constant
# sin(pi z) ~= k z (A - z^2)(B - z^2) minimax fit on [-1,1]; substituting
# z = 2r gives sin(2pi r) ~= KP * r (AQ - r^2)(BQ - r^2) on r in [-.5,.5].
AQ = 0.2512187245830011
BQ = 0.4600290215280054
KP = 53.520624390078666
# One fp32 ulp below 2*pi so |r|<=0.5 keeps Sin's argument inside [-pi,pi].
SCALE_SIN = float(np.nextafter(np.float32(2 * np.pi), np.float32(0)))

ND, NH = 23, 9  # DVE-path tiles vs Act+Pool+PE helper tiles


HELPER_TILES = (5, 7, 10, 13, 16, 19, 22, 25, 30)  # tuned by search


def _make_paths(nd=ND, nh=NH):
    """Helper-tile placement: denser early (while the DVE pipeline ramps),
    spacing 3 later; first/last tiles stay on the short DVE path.  The
    exact positions were hill-climbed against TimelineSim."""
    assert nd + nh == NT and nh == len(HELPER_TILES)
    out = ["d"] * NT
    for p in HELPER_TILES:
        out[p] = "h"
    return out


def _make_sine_op():
    """Custom DVE op: r = t - rne(t) (C0 = 1.5*2^23), out = r(C1-r^2)(C2-r^2).

    Registered into concourse.dve_ops at import time (idempotent)."""
    name = "SINE_QUINTIC_ANT"
    for o in dve_ops.OPS:
        if o.name == name:
            return o
    t = Src0
    u0 = t + C0
    u1 = u0 - C0
    r = t - u1
    s = r * r
    body = (r * (C1 - s)) * (C2 - s)

    def ref(in0, in1, s0, s1, imm2):
        tt = in0.astype(np.float32)
        u1 = ((tt + np.float32(s0)) - np.float32(s0)).astype(np.float32)
        r = (tt - u1).astype(np.float32)
        s = (r * r).astype(np.float32)
        return (
            (r * (np.float32(s1) - s)).astype(np.float32) * (np.float32(imm2) - s)
        ).astype(np.float32)

    spec = Spec(body=body, reference=ref)
    opcode = dve_ops._CUSTOM_DVE_ROW_BASE + len(dve_ops.OPS)
    dve_ops._SUB_OPCODE_FOR_NAME[name] = opcode
    shas = {}
    for ver in ("v3", "v4"):
        tmp = DveOpSpec(
            name=name, opcode=opcode, uops=lower(spec, ver=ver), rd1_en=False
        )
        shas[ver] = tmp.sha(ver)
    op = dve_ops.DveOp(name, spec, subdim=False, uops_sha=shas)
    dve_ops.OPS.append(op)
    dve_ops.CUSTOM_DVE_SPECS[name] = spec
    return op


def build(paths=None, prewarm=47, order=0, lag_s2=2, lag_mm2=5, gbufs=7, hwide=False, fphbufs=3, u0bufs=20, ubufs=20, dve_u=(), head_opt=0, interleave_s2=0, split_copy=0, head2=0, head3=0, wave_order=1, hhalf=2, dprio=0, mm2prio=0, tail_half=0):
    paths = paths or _make_paths()
    sine_op = _make_sine_op()
    Ident = mybir.ActivationFunctionType.Identity
    Sin = mybir.ActivationFunctionType.Sin
    nc = bacc.Bacc("TRN2", target_bir_lowering=False, debug=False, num_devices=NCORES)

    if head3:
        # One combined input tensor [wft tiles 0-1 | xt | wft tiles 2-31] so
        # the FIRST DMA (HWDGE issues serialize at ~625ns each) carries
        # everything tiles 0/1 need.
        xtw_d = nc.dram_tensor("xtw", [DA, 2 * P + BS + (NT - 2) * P], f32,
                               kind="ExternalInput").ap()
    else:
        xt_d = nc.dram_tensor("xt", [DA, BS], f32, kind="ExternalInput").ap()
        wft_d = nc.dram_tensor("wft", [DA, N], f32, kind="ExternalInput").ap()
    consts_d = nc.dram_tensor("consts", [P, P + NT * M], bf16, kind="ExternalInput").ap()
    out_d = nc.dram_tensor("out", [P, NBG, M], f32, kind="ExternalOutput").ap()

    with tile.TileContext(nc) as tc:
        with (
            tc.tile_pool(name="singles", bufs=1) as singles,
            tc.tile_pool(name="gpool", bufs=gbufs) as gpool,
            tc.tile_pool(name="u0pool", bufs=u0bufs) as u0pool,
            tc.tile_pool(name="upool", bufs=ubufs) as upool,
            tc.tile_pool(name="fpd", bufs=2, space="PSUM") as fpd,
            tc.tile_pool(name="fph", bufs=fphbufs, space="PSUM") as fph,
            tc.tile_pool(name="accpool", bufs=1, space="PSUM") as accpool,
        ):
            # Preload: few large DMAs split across the SP/Act hardware DGEs
            # and the Pool software DGE, ordered so the first MM1's inputs
            # (wft chunk 0, xt) land earliest.
            if head3:
                XW = 2 * P + BS  # start of wft tiles 2..31 in xtw
                xtw_sb = singles.tile([DA, 2 * P + BS + (NT - 2) * P], f32r,
                                      tag="xtw_sb")
            else:
                xt_sb = singles.tile([DA, BS], f32r, tag="xt_sb")
                wft_sb = singles.tile([DA, N], f32r, tag="wft_sb")
            consts_sb = singles.tile([P, P + NT * M], bf16, tag="consts")
            sinkp_sb = singles.tile([P, 1], f32, tag="sinkp")
            # dummy Sin as the first Act instruction pins the trig act-func
            # table once; Identity/Copy/memset_zero live in the same set.
            # Input is the framework's preamble-initialized const-zero AP.
            nc.scalar.activation(
                sinkp_sb[:], nc.const_aps.tensor(0.0, (P, 1)), Sin, scale=SCALE_SIN
            )
            if head3:
                # slivers01+xt0 | xt1 | consts (SWDGE, parallel issue) |
                # wft 2-7 | wft 8-19 | wft 20-31
                nc.sync.dma_start(xtw_sb[:, :2 * P + FREE],
                                  xtw_d[:, :2 * P + FREE].bitcast(f32r))
                nc.sync.dma_start(xtw_sb[:, 2 * P + FREE:XW],
                                  xtw_d[:, 2 * P + FREE:XW].bitcast(f32r))
                nc.gpsimd.dma_start(consts_sb, consts_d)
                nc.sync.dma_start(xtw_sb[:, XW:XW + 6 * P],
                                  xtw_d[:, XW:XW + 6 * P].bitcast(f32r))
                nc.sync.dma_start(xtw_sb[:, XW + 6 * P:XW + 18 * P],
                                  xtw_d[:, XW + 6 * P:XW + 18 * P].bitcast(f32r))
                nc.sync.dma_start(xtw_sb[:, XW + 18 * P:],
                                  xtw_d[:, XW + 18 * P:].bitcast(f32r))
            elif head2:
                # xt chunk 0 first (rhs of every tile's first MM1), then the
                # wft sliver for the first tiles, then the rest; first two
                # D-tiles run at half granularity so DVE starts ~1.4us sooner.
                nc.sync.dma_start(xt_sb[:, :FREE], xt_d[:, :FREE].bitcast(f32r))
                nc.scalar.dma_start(wft_sb[:, :FREE], wft_d[:, :FREE].bitcast(f32r))
                nc.sync.dma_start(xt_sb[:, FREE:], xt_d[:, FREE:].bitcast(f32r))
                nc.scalar.dma_start(consts_sb, consts_d)
                nc.sync.dma_start(
                    wft_sb[:, FREE:4 * FREE], wft_d[:, FREE:4 * FREE].bitcast(f32r)
                )
                nc.sync.dma_start(wft_sb[:, 4 * FREE:], wft_d[:, 4 * FREE:].bitcast(f32r))
            elif head_opt:
                # tile 0 needs only a 128-col sliver of wft; land it first
                nc.gpsimd.dma_start(xt_sb[:, :FREE], xt_d[:, :FREE].bitcast(f32r))
                nc.sync.dma_start(wft_sb[:, :P], wft_d[:, :P].bitcast(f32r))
                nc.sync.dma_start(xt_sb[:, FREE:], xt_d[:, FREE:].bitcast(f32r))
                nc.scalar.dma_start(consts_sb, consts_d)
                nc.sync.dma_start(wft_sb[:, P:FREE], wft_d[:, P:FREE].bitcast(f32r))
                nc.sync.dma_start(
                    wft_sb[:, FREE:4 * FREE], wft_d[:, FREE:4 * FREE].bitcast(f32r)
                )
                nc.sync.dma_start(wft_sb[:, 4 * FREE:], wft_d[:, 4 * FREE:].bitcast(f32r))
            else:
                nc.sync.dma_start(wft_sb[:, :FREE], wft_d[:, :FREE].bitcast(f32r))
                nc.gpsimd.dma_start(xt_sb, xt_d[:].bitcast(f32r))
                nc.scalar.dma_start(consts_sb, consts_d)
                nc.sync.dma_start(
                    wft_sb[:, FREE:4 * FREE], wft_d[:, FREE:4 * FREE].bitcast(f32r)
                )
                nc.sync.dma_start(
                    wft_sb[:, 4 * FREE:], wft_d[:, 4 * FREE:].bitcast(f32r)
                )
            if head3:
                xt_tiles = [xtw_sb[:, 2 * P + j * FREE:2 * P + (j + 1) * FREE]
                            for j in range(2)]

                def wsl(t):
                    if t < 2:
                        return xtw_sb[:, t * P:(t + 1) * P]
                    return xtw_sb[:, XW + (t - 2) * P:XW + (t - 1) * P]
            else:
                xt_tiles = [xt_sb[:, j * FREE:(j + 1) * FREE] for j in range(2)]
                wft_tiles = [wft_sb[:, c * FREE:(c + 1) * FREE] for c in range(8)]

                def wsl(t):
                    return wft_tiles[t // 4][:, (t % 4) * P:(t % 4 + 1) * P]
            negi_sb = consts_sb[:, :P]
            wps_sb = consts_sb[:, P:]
            magic_sb = singles.tile([P, 1], f32, tag="magicsb")
            nc.gpsimd.memset(magic_sb[:], MAGIC)
            # PE prewarm: a train of tiny matmuls on memset zeros keeps the PE
            # busy from t~0.4us, so the first real MM1s are decoded in the
            # MID p-state (and later ones at full speed) instead of LOW.
            warm_sb = singles.tile([P, 32], bf16, tag="warmsb")
            nc.gpsimd.memset(warm_sb[:], 0.0)

            acc = accpool.tile([P, NBG * M], f32)
            for _ in range(prewarm):
                nc.tensor.matmul(
                    acc[:32, :32], lhsT=warm_sb[:], rhs=warm_sb[:],
                    start=True, stop=True, skip_group_check=True,
                )

            fps_by_t = {}
            g_by_t = {}
            u0_by_t = {}

            from contextlib import nullcontext

            def emit_mm1(t):
                lhsT = wsl(t)
                if paths[t] == "d":
                    fps = fpd.tile([P, BS], f32)
                    fps_by_t[t] = fps
                    with tc.high_priority(offset=dprio) if dprio else nullcontext():
                        for j in range(2):
                            nc.tensor.matmul(
                                fps[:, j * FREE:(j + 1) * FREE],
                                lhsT=lhsT,
                                rhs=xt_tiles[j][:],
                                start=True,
                                stop=True,
                            )
                elif hwide:
                    fps = fph.tile([P, BS], f32)
                    fps_by_t[t] = fps
                    for j in range(2):
                        nc.tensor.matmul(
                            fps[:, j * FREE:(j + 1) * FREE],
                            lhsT=lhsT,
                            rhs=xt_tiles[j][:],
                            start=True,
                            stop=False,
                        )
                else:
                    halves = []
                    for j in range(2):
                        fh = fph.tile([P, FREE], f32)
                        halves.append(fh)
                        nc.tensor.matmul(
                            fh[:],
                            lhsT=lhsT,
                            rhs=xt_tiles[j][:],
                            start=True,
                            stop=False,
                        )
                    fps_by_t[t] = halves

            def emit_stage1(t):
                # produce either g (path d) or u0 halves (path h) from psum
                if paths[t] == "d":
                    g = gpool.tile([P, BS], bf16)
                    g_by_t[t] = g
                    if tail_half and t == NT - 1:
                        # last tile in halves: MM2s for batch-half 0 overlap
                        # the second half-op (subtile deps), shortening the
                        # post-DVE tail chain
                        for j in range(2):
                            sl = slice(j * FREE, (j + 1) * FREE)
                            nc.vector._custom_dve(
                                sine_op, out=g[:, sl], in0=fps_by_t[t][:, sl],
                                s0=MAGIC, s1=AQ, imm2=BQ
                            )
                    elif head_opt and t < head_opt:
                        for j in range(2):
                            sl = slice(j * FREE, (j + 1) * FREE)
                            nc.vector._custom_dve(
                                sine_op, out=g[:, sl], in0=fps_by_t[t][:, sl],
                                s0=MAGIC, s1=AQ, imm2=BQ
                            )
                    else:
                        nc.vector._custom_dve(
                            sine_op, out=g[:], in0=fps_by_t[t][:],
                            s0=MAGIC, s1=AQ, imm2=BQ
                        )
                elif hwide:
                    u0 = u0pool.tile([P, BS], f32)
                    nc.scalar.activation(
                        u0[:], fps_by_t[t][:], Ident, bias=magic_sb[:]
                    )
                    u = upool.tile([P, BS], bf16)
                    nc.gpsimd.tensor_scalar(
                        out=u[:], in0=u0[:], scalar1=MAGIC, scalar2=None,
                        op0=mybir.AluOpType.subtract,
                    )
                    u0_by_t[t] = u
                elif t in dve_u:
                    # DVE computes u = (t+magic)-magic directly from PSUM in
                    # one pass, relieving Act+Pool for this tile
                    us = []
                    for j in range(2):
                        u = upool.tile([P, FREE], bf16)
                        us.append(u)
                        nc.vector.tensor_scalar(
                            out=u[:], in0=fps_by_t[t][j][:],
                            scalar1=MAGIC, scalar2=MAGIC,
                            op0=mybir.AluOpType.add, op1=mybir.AluOpType.subtract,
                        )
                    u0_by_t[t] = us
                else:
                    us = []
                    for j in range(2):
                        u0 = u0pool.tile([P, FREE], f32)
                        nc.scalar.activation(
                            u0[:], fps_by_t[t][j][:], Ident, bias=magic_sb[:]
                        )
                        u = upool.tile([P, FREE], bf16)
                        us.append(u)
                        nc.gpsimd.tensor_scalar(
                            out=u[:], in0=u0[:], scalar1=MAGIC, scalar2=None,
                            op0=mybir.AluOpType.subtract,
                        )
                    u0_by_t[t] = us

            def emit_stage2(t):
                # path h: PE subtract (t - rne(t) -> r in psum), then Act Sin
                if paths[t] == "d":
                    return
                g = gpool.tile([P, BS], bf16)
                g_by_t[t] = g
                if interleave_s2 and not hwide:
                    # per-half sub->sin interleave: Act starts each Sin as
                    # soon as its own half's subtract lands
                    for j in range(2):
                        nc.tensor.matmul(
                            fps_by_t[t][j][:],
                            lhsT=negi_sb[:],
                            rhs=u0_by_t[t][j][:],
                            start=False,
                            stop=True,
                        )
                        nc.scalar.activation(
                            g[:, j * FREE:(j + 1) * FREE],
                            fps_by_t[t][j][:],
                            Sin,
                            scale=SCALE_SIN,
                        )
                elif hwide:
                    fps = fps_by_t[t]
                    u = u0_by_t[t]
                    for j in range(2):
                        nc.tensor.matmul(
                            fps[:, j * FREE:(j + 1) * FREE],
                            lhsT=negi_sb[:],
                            rhs=u[:, j * FREE:(j + 1) * FREE],
                            start=False,
                            stop=True,
                        )
                    nc.scalar.activation(g[:], fps[:], Sin, scale=SCALE_SIN)
                else:
                    for j in range(2):
                        nc.tensor.matmul(
                            fps_by_t[t][j][:],
                            lhsT=negi_sb[:],
                            rhs=u0_by_t[t][j][:],
                            start=False,
                            stop=True,
                        )
                    for j in range(2):
                        nc.scalar.activation(
                            g[:, j * FREE:(j + 1) * FREE],
                            fps_by_t[t][j][:],
                            Sin,
                            scale=SCALE_SIN,
                        )

            def emit_mm2(t):
                g = g_by_t[t]
                for bg in range(NBG):
                    nc.tensor.matmul(
                        acc[:, bg * M:(bg + 1) * M],
                        lhsT=g[:, bg * P:(bg + 1) * P],
                        rhs=wps_sb[:, t * M:(t + 1) * M],
                        start=(t == 0 and bg == 0),
                        stop=(t == NT - 1 and bg == NBG - 1),
                        skip_group_check=True,
                    )

            # Software-pipelined emission: stage2 lags 2 tiles, mm2 lags 3.
            # `order` picks the within-iteration emission order of the PE
            # work (accs/subs/mm1) to trade off which wait blocks the stream.
            def emit_iter(t):
                steps = {
                    0: ("acc", "sub", "mm1"),
                    1: ("mm1", "sub", "acc"),
                    2: ("mm1", "acc", "sub"),
                    3: ("sub", "mm1", "acc"),
                    4: ("acc", "mm1d", "sub", "mm1h"),
                    5: ("mm1d", "acc", "sub", "mm1h"),
                }[order]
                for s in steps:
                    if s == "acc" and 0 <= t - lag_mm2 < NT:
                        emit_mm2(t - lag_mm2)
                    elif s == "sub" and 0 <= t - lag_s2 < NT:
                        emit_stage2(t - lag_s2)
                    elif s == "mm1" and t < NT:
                        emit_mm1(t)
                        emit_stage1(t)
                    elif s == "mm1d" and t < NT and paths[t] == "d":
                        emit_mm1(t)
                        emit_stage1(t)
                    elif s == "mm1h" and t < NT and paths[t] != "d":
                        emit_mm1(t)
                        emit_stage1(t)

            t0 = 0
            if (head2 or head3) and hhalf:
                nh2 = min(int(hhalf), 2)
                # Half-granular head: tiles 0/1 (must be 'd') emitted as
                # [128,512] halves in two waves so the first DVE op runs as
                # soon as xt chunk 0 + the wft sliver land.  K=2 == fpd bufs;
                # larger K would block PE on a unit the second wave frees.
                assert all(paths[i] == "d" for i in range(nh2))
                t0 = nh2
                for t in range(nh2):
                    fps_by_t[t] = fpd.tile([P, BS], f32, name="fps", tag="fps")
                    g_by_t[t] = gpool.tile([P, BS], bf16, name="g", tag="g")
                if wave_order == 0:
                    hh = [(t, j) for j in (0, 1) for t in range(nh2)]
                else:
                    hh = [(t, j) for t in range(nh2) for j in (0, 1)]
                for t, j in hh:
                    sl = slice(j * FREE, (j + 1) * FREE)
                    nc.tensor.matmul(
                        fps_by_t[t][:, sl],
                        lhsT=wsl(t),
                        rhs=xt_tiles[j][:],
                        start=True,
                        stop=True,
                    )
                    nc.vector._custom_dve(
                        sine_op, out=g_by_t[t][:, sl], in0=fps_by_t[t][:, sl],
                        s0=MAGIC, s1=AQ, imm2=BQ
                    )
            for t in range(t0, NT + max(lag_s2, lag_mm2)):
                emit_iter(t)

            out_sb = singles.tile([P, NBG * M], f32, tag="outsb")
            if split_copy:
                half = NBG * M // 2
                nc.vector.tensor_copy(out=out_sb[:, :half], in_=acc[:, :half])
                nc.scalar.activation(
                    out_sb[:, half:], acc[:, half:],
                    mybir.ActivationFunctionType.Identity,
                )
            else:
                nc.vector.tensor_copy(out=out_sb[:], in_=acc[:])
            nc.sync.dma_start(out_d, out_sb[:])
    nc.compile()
    return nc


_NC = None
# Build knobs used by _get_nc()/run(); keep head3 here in sync with
# _prep_in_maps' input layout.  TimelineSim: 35667 ns (baseline 36440).
BUILD_KW = {"head3": 1, "gbufs": 8, "order": 3, "hhalf": 1, "lag_s2": 1}


def _get_nc():
    global _NC
    if _NC is None:
        _NC = build(**BUILD_KW)
    return _NC


def _prep_in_maps(x, Wf, bf, Wp, paths=None):
    paths = paths or BUILD_KW.get("paths") or _make_paths()
    scale = np.float64(np.sqrt(2.0 / N))
    inv2pi = np.float64(1.0) / (2.0 * np.pi)
    # [65, 4096]: rows 0-63 = (Wf/2pi)^T, row 64 = bf/2pi + 1/4 (cos->sin)
    wft = np.empty((DA, N), dtype=np.float32)
    wft[:D] = (Wf.astype(np.float64) * inv2pi).astype(np.float32).T
    wft[D] = (bf.astype(np.float64) * inv2pi + 0.25).astype(np.float32)
    # Wp scaled per tile: DVE-path tiles additionally absorb the quintic's
    # leading coefficient KP.  [128, NT, M] in bf16.
    wps64 = Wp.astype(np.float64).reshape(NT, P, M) * scale
    for t in range(NT):
        if paths[t] == "d":
            wps64[t] *= KP
    consts = np.empty((P, P + NT * M), dtype=ml_dtypes.bfloat16)
    consts[:, :P] = (-np.eye(P)).astype(ml_dtypes.bfloat16)
    consts[:, P:] = (
        np.ascontiguousarray(wps64.transpose(1, 0, 2))
        .reshape(P, NT * M)
        .astype(ml_dtypes.bfloat16)
    )
    head3 = BUILD_KW.get("head3", 0)
    in_maps = []
    for c in range(NCORES):
        xt = np.empty((DA, BS), dtype=np.float32)
        xt[:D] = x[c * BS:(c + 1) * BS].astype(np.float32).T
        xt[D] = 1.0
        if head3:
            xtw = np.empty((DA, 2 * P + BS + (NT - 2) * P), dtype=np.float32)
            xtw[:, :2 * P] = wft[:, :2 * P]
            xtw[:, 2 * P:2 * P + BS] = xt
            xtw[:, 2 * P + BS:] = wft[:, 2 * P:]
            in_maps.append({"xtw": xtw, "consts": consts})
        else:
            in_maps.append({"xt": xt, "wft": wft, "consts": consts})
    return in_maps


def run(x, Wf, bf, Wp, trace=False):
    nc = _get_nc()
    in_maps = _prep_in_maps(x, Wf, bf, Wp)
    res = bass_utils.run_bass_kernel_spmd(
        nc, in_maps, core_ids=list(range(NCORES)), trace=trace
    )
    out = np.empty((B, M), dtype=np.float32)
    for c in range(NCORES):
        # device out: [P, NBG, M] with out[bg*128+p, m] = dev[p, bg, m]
        dev = res.results[c]["out"]
        out[c * BS:(c + 1) * BS, :] = dev.transpose(1, 0, 2).reshape(BS, M)
    return out, res


def kernel(x, Wf, bf, Wp):
    x = np.asarray(x)
    Wf = np.asarray(Wf)
    bf = np.asarray(bf)
    Wp = np.asarray(Wp)
    out, _ = run(x, Wf, bf, Wp, trace=False)
    return out



# revision 24
# speedup vs baseline: 1.0135x; 1.0085x over previous
"""Trainium2 Bass kernel for nn_KernelMachine (random-feature kernel machine).

Computes out = (sqrt(2/N) * cos(x @ Wf^T + bf)) @ Wp on 8 NeuronCores,
data-parallel over the batch dimension (1024 rows/core), no collectives.

Per-core pipeline, per 128-wide tile of the N=4096 feature dim, with the
elementwise work load-balanced across DVE and the Activation engine:

  MM1 (TensorE, f32r): t = [x | 1] @ [Wf/2pi | bf']  -> PSUM (in turns)
      (bias and the cos->sin quarter-turn fold into the ones-row weights)
  then one of two per-tile paths to g ~ sin(2pi t):
   - path 'd' (DVE): one fused custom op on [128,1024]: r = t - rne(t) via
     the 1.5*2^23 magic constant, then the factored quintic r(A-r^2)(B-r^2)
     which is sin(2pi r)/K for K=53.52...  (K folds into this tile's Wp)
   - path 'h' (Act+Pool+PE, two [128,512] halves): Act u0 = t + 1.5*2^23
     (= magic + rne(t), SBUF), Pool u = u0 - magic (bf16, small ints),
     PE accumulates t - u into the same PSUM bank via a -I matmul, Act
     applies Sin.
  MM2 (TensorE, bf16, transposed): out[b,m] accumulated as 8 tiny
      [128b x 8m] matmuls per tile (lhsT = g columns, rhs = Wp tile);
      cost scales with the 8-wide moving dim instead of the 1024 batch.

PSUM: d-tiles 2 bufs x 2 banks, helper halves 3 bufs x 1 bank, acc 1 bank.

Scheduling: software-pipelined emission (subtract+Sin lag 2 tiles, MM2 lag
5) with ready-work-first ordering per engine stream; a PE "prewarm" train
of tiny matmuls keeps the tensor engine's p-state ramp warm so real
matmuls run at full clock; a leading dummy Sin pins the trig activation
table once.

Head (head3): HWDGE DMA issues serialize at ~625ns each and every
DMA-dependent start pays a 900ns semaphore-propagation penalty, so the
inputs ship as ONE combined tensor [wft tiles 0-1 | xt | wft tiles 2-31]
whose first chunk (xt half 0 + the first two weight slivers) is all tiles
0/1 need; tile 0 then runs at [128,512] half granularity so the first
DVE op starts ~3.9us (vs 4.5us), the earliest the DMA chain allows.
Helper tiles (5,7,...,30) and knobs hill-climbed against TimelineSim.
"""
import sys

if "/opt/trn_rl_repo" not in sys.path:
    sys.path.insert(0, "/opt/trn_rl_repo")

import ml_dtypes
import numpy as np

import concourse.bacc as bacc
import concourse.mybir as mybir
import concourse.tile as tile
from concourse import bass_utils
from concourse import dve_ops
from concourse.dve_spec import C0, C1, C2, Spec, Src0, lower
from concourse.dve_uop import DveOpSpec

# Problem shape (hardcoded per contest contract).
B = 8192
D = 64
DA = D + 1  # contraction augmented with a ones-row (bias)
N = 4096
M = 8
NCORES = 8
BS = B // NCORES  # 1024 batch rows per core
P = 128
NT = N // P  # 32 feature tiles
FREE = 512  # MM1 matmul moving free dim (one PSUM bank)
NBG = BS // P  # 8 batch groups for the transposed MM2

f32 = mybir.dt.float32
f32r = mybir.dt.float32r
f16 = mybir.dt.float16
bf16 = mybir.dt.bfloat16

MAGIC = float(1.5 * 2**23)  # fp32 round-to-nearest-int magic constant
# sin(pi z) ~= k z (A - z^2)(B - z^2) minimax fit on [-1,1]; substituting
# z = 2r gives sin(2pi r) ~= KP * r (AQ - r^2)(BQ - r^2) on r in [-.5,.5].
AQ = 0.2512187245830011
BQ = 0.4600290215280054
KP = 53.520624390078666
# One fp32 ulp below 2*pi so |r|<=0.5 keeps Sin's argument inside [-pi,pi].
SCALE_SIN = float(np.nextafter(np.float32(2 * np.pi), np.float32(0)))

ND, NH = 23, 9  # DVE-path tiles vs Act+Pool+PE helper tiles


HELPER_TILES = (5, 7, 10, 13, 16, 19, 22, 25, 30)  # tuned by search


def _make_paths(nd=ND, nh=NH):
    """Helper-tile placement: denser early (while the DVE pipeline ramps),
    spacing 3 later; first/last tiles stay on the short DVE path.  The
    exact positions were hill-climbed against TimelineSim."""
    assert nd + nh == NT and nh == len(HELPER_TILES)
    out = ["d"] * NT
    for p in HELPER_TILES:
        out[p] = "h"
    return out


def _make_sine_op():
    """Custom DVE op: r = t - rne(t) (C0 = 1.5*2^23), out = r(C1-r^2)(C2-r^2).

    Registered into concourse.dve_ops at import time (idempotent)."""
    name = "SINE_QUINTIC_ANT"
    for o in dve_ops.OPS:
        if o.name == name:
            return o
    t = Src0
    u0 = t + C0
    u1 = u0 - C0
    r = t - u1
    s = r * r
    body = (r * (C1 - s)) * (C2 - s)

    def ref(in0, in1, s0, s1, imm2):
        tt = in0.astype(np.float32)
        u1 = ((tt + np.float32(s0)) - np.float32(s0)).astype(np.float32)
        r = (tt - u1).astype(np.float32)
        s = (r * r).astype(np.float32)
        return (
            (r * (np.float32(s1) - s)).astype(np.float32) * (np.float32(imm2) - s)
        ).astype(np.float32)

    spec = Spec(body=body, reference=ref)
    opcode = dve_ops._CUSTOM_DVE_ROW_BASE + len(dve_ops.OPS)
    dve_ops._SUB_OPCODE_FOR_NAME[name] = opcode
    shas = {}
    for ver in ("v3", "v4"):
        tmp = DveOpSpec(
            name=name, opcode=opcode, uops=lower(spec, ver=ver), rd1_en=False
        )
        shas[ver] = tmp.sha(ver)
    op = dve_ops.DveOp(name, spec, subdim=False, uops_sha=shas)
    dve_ops.OPS.append(op)
    dve_ops.CUSTOM_DVE_SPECS[name] = spec
    return op


def build(paths=None, prewarm=47, order=0, lag_s2=2, lag_mm2=5, gbufs=7, hwide=False, fphbufs=3, u0bufs=20, ubufs=20, dve_u=(), head_opt=0, interleave_s2=0, split_copy=0, head2=0, head3=0, wave_order=1, hhalf=2, dprio=0, mm2prio=0, tail_half=0):
    paths = paths or _make_paths()
    sine_op = _make_sine_op()
    Ident = mybir.ActivationFunctionType.Identity
    Sin = mybir.ActivationFunctionType.Sin
    nc = bacc.Bacc("TRN2", target_bir_lowering=False, debug=False, num_devices=NCORES)

    if head3:
        # One combined input tensor [wft tiles 0-1 | xt | wft tiles 2-31] so
        # the FIRST DMA (HWDGE issues serialize at ~625ns each) carries
        # everything tiles 0/1 need.
        xtw_d = nc.dram_tensor("xtw", [DA, 2 * P + BS + (NT - 2) * P], f32,
                               kind="ExternalInput").ap()
    else:
        xt_d = nc.dram_tensor("xt", [DA, BS], f32, kind="ExternalInput").ap()
        wft_d = nc.dram_tensor("wft", [DA, N], f32, kind="ExternalInput").ap()
    consts_d = nc.dram_tensor("consts", [P, P + NT * M], bf16, kind="ExternalInput").ap()
    out_d = nc.dram_tensor("out", [P, NBG, M], f32, kind="ExternalOutput").ap()

    with tile.TileContext(nc) as tc:
        with (
            tc.tile_pool(name="singles", bufs=1) as singles,
            tc.tile_pool(name="gpool", bufs=gbufs) as gpool,
            tc.tile_pool(name="u0pool", bufs=u0bufs) as u0pool,
            tc.tile_pool(name="upool", bufs=ubufs) as upool,
            tc.tile_pool(name="fpd", bufs=2, space="PSUM") as fpd,
            tc.tile_pool(name="fph", bufs=fphbufs, space="PSUM") as fph,
            tc.tile_pool(name="accpool", bufs=1, space="PSUM") as accpool,
        ):
            # Preload: few large DMAs split across the SP/Act hardware DGEs
            # and the Pool software DGE, ordered so the first MM1's inputs
            # (wft chunk 0, xt) land earliest.
            if head3:
                XW = 2 * P + BS  # start of wft tiles 2..31 in xtw
                xtw_sb = singles.tile([DA, 2 * P + BS + (NT - 2) * P], f32r,
                                      tag="xtw_sb")
            else:
                xt_sb = singles.tile([DA, BS], f32r, tag="xt_sb")
                wft_sb = singles.tile([DA, N], f32r, tag="wft_sb")
            consts_sb = singles.tile([P, P + NT * M], bf16, tag="consts")
            sinkp_sb = singles.tile([P, 1], f32, tag="sinkp")
            # dummy Sin as the first Act instruction pins the trig act-func
            # table once; Identity/Copy/memset_zero live in the same set.
            # Input is the framework's preamble-initialized const-zero AP.
            nc.scalar.activation(
                sinkp_sb[:], nc.const_aps.tensor(0.0, (P, 1)), Sin, scale=SCALE_SIN
            )
            if head3:
                # slivers01+xt0 | xt1 | consts (SWDGE, parallel issue) |
                # wft 2-7 | wft 8-19 | wft 20-31
                nc.sync.dma_start(xtw_sb[:, :2 * P + FREE],
                                  xtw_d[:, :2 * P + FREE].bitcast(f32r))
                nc.sync.dma_start(xtw_sb[:, 2 * P + FREE:XW],
                                  xtw_d[:, 2 * P + FREE:XW].bitcast(f32r))
                nc.gpsimd.dma_start(consts_sb, consts_d)
                nc.sync.dma_start(xtw_sb[:, XW:XW + 6 * P],
                                  xtw_d[:, XW:XW + 6 * P].bitcast(f32r))
                nc.sync.dma_start(xtw_sb[:, XW + 6 * P:XW + 18 * P],
                                  xtw_d[:, XW + 6 * P:XW + 18 * P].bitcast(f32r))
                nc.sync.dma_start(xtw_sb[:, XW + 18 * P:],
                                  xtw_d[:, XW + 18 * P:].bitcast(f32r))
            elif head2:
                # xt chunk 0 first (rhs of every tile's first MM1), then the
                # wft sliver for the first tiles, then the rest; first two
                # D-tiles run at half granularity so DVE starts ~1.4us sooner.
                nc.sync.dma_start(xt_sb[:, :FREE], xt_d[:, :FREE].bitcast(f32r))
                nc.scalar.dma_start(wft_sb[:, :FREE], wft_d[:, :FREE].bitcast(f32r))
                nc.sync.dma_start(xt_sb[:, FREE:], xt_d[:, FREE:].bitcast(f32r))
                nc.scalar.dma_start(consts_sb, consts_d)
                nc.sync.dma_start(
                    wft_sb[:, FREE:4 * FREE], wft_d[:, FREE:4 * FREE].bitcast(f32r)
                )
                nc.sync.dma_start(wft_sb[:, 4 * FREE:], wft_d[:, 4 * FREE:].bitcast(f32r))
            elif head_opt:
                # tile 0 needs only a 128-col sliver of wft; land it first
                nc.gpsimd.dma_start(xt_sb[:, :FREE], xt_d[:, :FREE].bitcast(f32r))
                nc.sync.dma_start(wft_sb[:, :P], wft_d[:, :P].bitcast(f32r))
                nc.sync.dma_start(xt_sb[:, FREE:], xt_d[:, FREE:].bitcast(f32r))
                nc.scalar.dma_start(consts_sb, consts_d)
                nc.sync.dma_start(wft_sb[:, P:FREE], wft_d[:, P:FREE].bitcast(f32r))
                nc.sync.dma_start(
                    wft_sb[:, FREE:4 * FREE], wft_d[:, FREE:4 * FREE].bitcast(f32r)
                )
                nc.sync.dma_start(wft_sb[:, 4 * FREE:], wft_d[:, 4 * FREE:].bitcast(f32r))
            else:
                nc.sync.dma_start(wft_sb[:, :FREE], wft_d[:, :FREE].bitcast(f32r))
                nc.gpsimd.dma_start(xt_sb, xt_d[:].bitcast(f32r))
                nc.scalar.dma_start(consts_sb, consts_d)
                nc.sync.dma_start(
                    wft_sb[:, FREE:4 * FREE], wft_d[:, FREE:4 * FREE].bitcast(f32r)
                )
                nc.sync.dma_start(
                    wft_sb[:, 4 * FREE:], wft_d[:, 4 * FREE:].bitcast(f32r)
                )
            if head3:
                xt_tiles = [xtw_sb[:, 2 * P + j * FREE:2 * P + (j + 1) * FREE]
                            for j in range(2)]

                def wsl(t):
                    if t < 2:
                        return xtw_sb[:, t * P:(t + 1) * P]
                    return xtw_sb[:, XW + (t - 2) * P:XW + (t - 1) * P]
            else:
                xt_tiles = [xt_sb[:, j * FREE:(j + 1) * FREE] for j in range(2)]
                wft_tiles = [wft_sb[:, c * FREE:(c + 1) * FREE] for c in range(8)]

                def wsl(t):
                    return wft_tiles[t // 4][:, (t % 4) * P:(t % 4 + 1) * P]
            negi_sb = consts_sb[:, :P]
            wps_sb = consts_sb[:, P:]
            magic_sb = singles.tile([P, 1], f32, tag="magicsb")
            nc.gpsimd.memset(magic_sb[:], MAGIC)
            # PE prewarm: a train of tiny matmuls on memset zeros keeps the PE
            # busy from t~0.4us, so the first real MM1s are decoded in the
            # MID p-state (and later ones at full speed) instead of LOW.
            warm_sb = singles.tile([P, 32], bf16, tag="warmsb")
            nc.gpsimd.memset(warm_sb[:], 0.0)

            acc = accpool.tile([P, NBG * M], f32)
            for _ in range(prewarm):
                nc.tensor.matmul(
                    acc[:32, :32], lhsT=warm_sb[:], rhs=warm_sb[:],
                    start=True, stop=True, skip_group_check=True,
                )

            fps_by_t = {}
            g_by_t = {}
            u0_by_t = {}

            from contextlib import nullcontext

            def emit_mm1(t):
                lhsT = wsl(t)
                if paths[t] == "d":
                    fps = fpd.tile([P, BS], f32)
                    fps_by_t[t] = fps
                    with tc.high_priority(offset=dprio) if dprio else nullcontext():
                        for j in range(2):
                            nc.tensor.matmul(
                                fps[:, j * FREE:(j + 1) * FREE],
                                lhsT=lhsT,
                                rhs=xt_tiles[j][:],
                                start=True,
                                stop=True,
                            )
                elif hwide:
                    fps = fph.tile([P, BS], f32)
                    fps_by_t[t] = fps
                    for j in range(2):
                        nc.tensor.matmul(
                            fps[:, j * FREE:(j + 1) * FREE],
                            lhsT=lhsT,
                            rhs=xt_tiles[j][:],
                            start=True,
                            stop=False,
                        )
                else:
                    halves = []
                    for j in range(2):
                        fh = fph.tile([P, FREE], f32)
                        halves.append(fh)
                        nc.tensor.matmul(
                            fh[:],
                            lhsT=lhsT,
                            rhs=xt_tiles[j][:],
                            start=True,
                            stop=False,
                        )
                    fps_by_t[t] = halves

            def emit_stage1(t):
                # produce either g (path d) or u0 halves (path h) from psum
                if paths[t] == "d":
                    g = gpool.tile([P, BS], bf16)
                    g_by_t[t] = g
                    if tail_half and t == NT - 1:
                        # last tile in halves: MM2s for batch-half 0 overlap
                        # the second half-op (subtile deps), shortening the
                        # post-DVE tail chain
                        for j in range(2):
                            sl = slice(j * FREE, (j + 1) * FREE)
                            nc.vector._custom_dve(
                                sine_op, out=g[:, sl], in0=fps_by_t[t][:, sl],
                                s0=MAGIC, s1=AQ, imm2=BQ
                            )
                    elif head_opt and t < head_opt:
                        for j in range(2):
                            sl = slice(j * FREE, (j + 1) * FREE)
                            nc.vector._custom_dve(
                                sine_op, out=g[:, sl], in0=fps_by_t[t][:, sl],
                                s0=MAGIC, s1=AQ, imm2=BQ
                            )
                    else:
                        nc.vector._custom_dve(
                            sine_op, out=g[:], in0=fps_by_t[t][:],
                            s0=MAGIC, s1=AQ, imm2=BQ
                        )
                elif hwide:
                    u0 = u0pool.tile([P, BS], f32)
                    nc.scalar.activation(
                        u0[:], fps_by_t[t][:], Ident, bias=magic_sb[:]
                    )
                    u = upool.tile([P, BS], bf16)
                    nc.gpsimd.tensor_scalar(
                        out=u[:], in0=u0[:], scalar1=MAGIC, scalar2=None,
                        op0=mybir.AluOpType.subtract,
                    )
                    u0_by_t[t] = u
                elif t in dve_u:
                    # DVE computes u = (t+magic)-magic directly from PSUM in
                    # one pass, relieving Act+Pool for this tile
                    us = []
                    for j in range(2):
                        u = upool.tile([P, FREE], bf16)
                        us.append(u)
                        nc.vector.tensor_scalar(
                            out=u[:], in0=fps_by_t[t][j][:],
                            scalar1=MAGIC, scalar2=MAGIC,
                            op0=mybir.AluOpType.add, op1=mybir.AluOpType.subtract,
                        )
                    u0_by_t[t] = us
                else:
                    us = []
                    for j in range(2):
                        u0 = u0pool.tile([P, FREE], f32)
                        nc.scalar.activation(
                            u0[:], fps_by_t[t][j][:], Ident, bias=magic_sb[:]
                        )
                        u = upool.tile([P, FREE], bf16)
                        us.append(u)
                        nc.gpsimd.tensor_scalar(
                            out=u[:], in0=u0[:], scalar1=MAGIC, scalar2=None,
                            op0=mybir.AluOpType.subtract,
                        )
                    u0_by_t[t] = us

            def emit_stage2(t):
                # path h: PE subtract (t - rne(t) -> r in psum), then Act Sin
                if paths[t] == "d":
                    return
                g = gpool.tile([P, BS], bf16)
                g_by_t[t] = g
                if interleave_s2 and not hwide:
                    # per-half sub->sin interleave: Act starts each Sin as
                    # soon as its own half's subtract lands
                    for j in range(2):
                        nc.tensor.matmul(
                            fps_by_t[t][j][:],
                            lhsT=negi_sb[:],
                            rhs=u0_by_t[t][j][:],
                            start=False,
                            stop=True,
                        )
                        nc.scalar.activation(
                            g[:, j * FREE:(j + 1) * FREE],
                            fps_by_t[t][j][:],
                            Sin,
                            scale=SCALE_SIN,
                        )
                elif hwide:
                    fps = fps_by_t[t]
                    u = u0_by_t[t]
                    for j in range(2):
                        nc.tensor.matmul(
                            fps[:, j * FREE:(j + 1) * FREE],
                            lhsT=negi_sb[:],
                            rhs=u[:, j * FREE:(j + 1) * FREE],
                            start=False,
                            stop=True,
                        )
                    nc.scalar.activation(g[:], fps[:], Sin, scale=SCALE_SIN)
                else:
                    for j in range(2):
                        nc.tensor.matmul(
                            fps_by_t[t][j][:],
                            lhsT=negi_sb[:],
                            rhs=u0_by_t[t][j][:],
                            start=False,
                            stop=True,
                        )
                    for j in range(2):
                        nc.scalar.activation(
                            g[:, j * FREE:(j + 1) * FREE],
                            fps_by_t[t][j][:],
                            Sin,
                            scale=SCALE_SIN,
                        )

            def emit_mm2(t):
                g = g_by_t[t]
                for bg in range(NBG):
                    nc.tensor.matmul(
                        acc[:, bg * M:(bg + 1) * M],
                        lhsT=g[:, bg * P:(bg + 1) * P],
                        rhs=wps_sb[:, t * M:(t + 1) * M],
                        start=(t == 0 and bg == 0),
                        stop=(t == NT - 1 and bg == NBG - 1),
                        skip_group_check=True,
                    )

            # Software-pipelined emission: stage2 lags 2 tiles, mm2 lags 3.
            # `order` picks the within-iteration emission order of the PE
            # work (accs/subs/mm1) to trade off which wait blocks the stream.
            def emit_iter(t):
                steps = {
                    0: ("acc", "sub", "mm1"),
                    1: ("mm1", "sub", "acc"),
                    2: ("mm1", "acc", "sub"),
                    3: ("sub", "mm1", "acc"),
                    4: ("acc", "mm1d", "sub", "mm1h"),
                    5: ("mm1d", "acc", "sub", "mm1h"),
                }[order]
                for s in steps:
                    if s == "acc" and 0 <= t - lag_mm2 < NT:
                        emit_mm2(t - lag_mm2)
                    elif s == "sub" and 0 <= t - lag_s2 < NT:
                        emit_stage2(t - lag_s2)
                    elif s == "mm1" and t < NT:
                        emit_mm1(t)
                        emit_stage1(t)
                    elif s == "mm1d" and t < NT and paths[t] == "d":
                        emit_mm1(t)
                        emit_stage1(t)
                    elif s == "mm1h" and t < NT and paths[t] != "d":
                        emit_mm1(t)
                        emit_stage1(t)

            t0 = 0
            if (head2 or head3) and hhalf:
                nh2 = min(int(hhalf), 2)
                # Half-granular head: tiles 0/1 (must be 'd') emitted as
                # [128,512] halves in two waves so the first DVE op runs as
                # soon as xt chunk 0 + the wft sliver land.  K=2 == fpd bufs;
                # larger K would block PE on a unit the second wave frees.
                assert all(paths[i] == "d" for i in range(nh2))
                t0 = nh2
                for t in range(nh2):
                    fps_by_t[t] = fpd.tile([P, BS], f32, name="fps", tag="fps")
                    g_by_t[t] = gpool.tile([P, BS], bf16, name="g", tag="g")
                if wave_order == 0:
                    hh = [(t, j) for j in (0, 1) for t in range(nh2)]
                else:
                    hh = [(t, j) for t in range(nh2) for j in (0, 1)]
                for t, j in hh:
                    sl = slice(j * FREE, (j + 1) * FREE)
                    nc.tensor.matmul(
                        fps_by_t[t][:, sl],
                        lhsT=wsl(t),
                        rhs=xt_tiles[j][:],
                        start=True,
                        stop=True,
                    )
                    nc.vector._custom_dve(
                        sine_op, out=g_by_t[t][:, sl], in0=fps_by_t[t][:, sl],
                        s0=MAGIC, s1=AQ, imm2=BQ
                    )
            for t in range(t0, NT + max(lag_s2, lag_mm2)):
                emit_iter(t)

            out_sb = singles.tile([P, NBG * M], f32, tag="outsb")
            if split_copy:
                half = NBG * M // 2
                nc.vector.tensor_copy(out=out_sb[:, :half], in_=acc[:, :half])
                nc.scalar.activation(
                    out_sb[:, half:], acc[:, half:],
                    mybir.ActivationFunctionType.Identity,
                )
            else:
                nc.vector.tensor_copy(out=out_sb[:], in_=acc[:])
            nc.sync.dma_start(out_d, out_sb[:])
    nc.compile()
    return nc


_NC = None
# Build knobs used by _get_nc()/run(); keep head3 here in sync with
# _prep_in_maps' input layout.  TimelineSim: 35367 ns (baseline 36440).
BUILD_KW = {"head3": 1, "gbufs": 8, "order": 3, "hhalf": 1, "lag_s2": 1, "prewarm": 60}


def _get_nc():
    global _NC
    if _NC is None:
        _NC = build(**BUILD_KW)
    return _NC


def _prep_in_maps(x, Wf, bf, Wp, paths=None):
    paths = paths or BUILD_KW.get("paths") or _make_paths()
    scale = np.float64(np.sqrt(2.0 / N))
    inv2pi = np.float64(1.0) / (2.0 * np.pi)
    # [65, 4096]: rows 0-63 = (Wf/2pi)^T, row 64 = bf/2pi + 1/4 (cos->sin)
    wft = np.empty((DA, N), dtype=np.float32)
    wft[:D] = (Wf.astype(np.float64) * inv2pi).astype(np.float32).T
    wft[D] = (bf.astype(np.float64) * inv2pi + 0.25).astype(np.float32)
    # Wp scaled per tile: DVE-path tiles additionally absorb the quintic's
    # leading coefficient KP.  [128, NT, M] in bf16.
    wps64 = Wp.astype(np.float64).reshape(NT, P, M) * scale
    for t in range(NT):
        if paths[t] == "d":
            wps64[t] *= KP
    consts = np.empty((P, P + NT * M), dtype=ml_dtypes.bfloat16)
    consts[:, :P] = (-np.eye(P)).astype(ml_dtypes.bfloat16)
    consts[:, P:] = (
        np.ascontiguousarray(wps64.transpose(1, 0, 2))
        .reshape(P, NT * M)
        .astype(ml_dtypes.bfloat16)
    )
    head3 = BUILD_KW.get("head3", 0)
    in_maps = []
    for c in range(NCORES):
        xt = np.empty((DA, BS), dtype=np.float32)
        xt[:D] = x[c * BS:(c + 1) * BS].astype(np.float32).T
        xt[D] = 1.0
        if head3:
            xtw = np.empty((DA, 2 * P + BS + (NT - 2) * P), dtype=np.float32)
            xtw[:, :2 * P] = wft[:, :2 * P]
            xtw[:, 2 * P:2 * P + BS] = xt
            xtw[:, 2 * P + BS:] = wft[:, 2 * P:]
            in_maps.append({"xtw": xtw, "consts": consts})
        else:
            in_maps.append({"xt": xt, "wft": wft, "consts": consts})
    return in_maps


def run(x, Wf, bf, Wp, trace=False):
    nc = _get_nc()
    in_maps = _prep_in_maps(x, Wf, bf, Wp)
    res = bass_utils.run_bass_kernel_spmd(
        nc, in_maps, core_ids=list(range(NCORES)), trace=trace
    )
    out = np.empty((B, M), dtype=np.float32)
    for c in range(NCORES):
        # device out: [P, NBG, M] with out[bg*128+p, m] = dev[p, bg, m]
        dev = res.results[c]["out"]
        out[c * BS:(c + 1) * BS, :] = dev.transpose(1, 0, 2).reshape(BS, M)
    return out, res


def kernel(x, Wf, bf, Wp):
    x = np.asarray(x)
    Wf = np.asarray(Wf)
    bf = np.asarray(bf)
    Wp = np.asarray(Wp)
    out, _ = run(x, Wf, bf, Wp, trace=False)
    return out

